# revision 36
# baseline (speedup 1.0000x reference)
"""Trainium2 Bass kernel for MixerDiffAttention (differential attention).

Sharding: tensor-parallel over the 8 (n_head//2) head groups across 8 cores.
Each core computes QKV projections for its head group, both differential
attention branches, the normalized combination y1 - lambda*y2, and its head's
partial product with the row-sharded c_proj. The host sums the 8 partial
outputs.

v3 notes:
  - All tensors fp16 on-chip; every DMA is per-partition contiguous.
  - Stage B works in groups of 4 t-blocks: QKV matmuls -> ACT stages raw q/k
    to SBUF -> DVE computes the rms scale with a Newton rsqrt (linear init,
    clipped, 3 iterations) so the only ACT table set ever loaded is exp's ->
    batched rotary -> one batched DMA xbar transpose per group for qT/kT.
  - scores exp'ed with bias -8 (|score*scale| <= 8 after rms norm) into fp16.
  - softmax denominators: group 0 accumulates on PE (ones-matmul per s-block),
    group 1 on DVE in fp16 (2x packed); one ones-matmul per t-chunk reduces
    group 1 across partitions.
  - partial projection uses ycomb chunks as the stationary operand, giving
    [t, i]-layout outputs; PSUM evacuation alternates DVE/ACT; projection
    PSUM is double-buffered via the pd pool ring.
"""

import os
import sys

import numpy as np

for _p in ("/opt/trn_rl_repo", "/root/.axon_site/_ro/trn_rl_repo"):
    if os.path.isdir(_p) and _p not in sys.path:
        sys.path.insert(0, _p)

import concourse.bass as bass
import concourse.mybir as mybir
import concourse.tile as tile
from concourse import bacc
from concourse.bass import ds, ts
from concourse.bass_utils import run_bass_kernel_spmd

FP16 = mybir.dt.float16
F32 = mybir.dt.float32
AF = mybir.ActivationFunctionType
ALU = mybir.AluOpType

N_HEAD = 16
D = 1024
HD = 64  # head dim
T = 2048
NCORES = 8
TB = T // 128  # 16 t-blocks
KC = D // 128  # 8 contraction chunks
NTC = T // 512  # 4 t-chunks of 512
LAMBDA_INIT = 0.8 - 0.6 * float(np.exp(-0.3 * 1))
EPS = float(np.finfo(np.float32).eps)
SCALE = 1.0 / 8.0  # 1/sqrt(64)

_CACHE = {}


def _build_program(lam: float) -> bass.Bass:
    nc = bacc.Bacc("TRN2", target_bir_lowering=False, debug=False)

    xd = nc.declare_dram_parameter("x", [128, NTC * KC * 512], FP16, isOutput=False)
    wqkvd = nc.declare_dram_parameter("wqkv", [128, KC * 384], FP16, isOutput=False)
    wppd = nc.declare_dram_parameter("wpp", [128, D], FP16, isOutput=False)
    costd = nc.declare_dram_parameter("cost", [128, 2 * TB * 32], FP16, isOutput=False)
    diagd = nc.declare_dram_parameter("diag", [128, 128], FP16, isOutput=False)
    outd = nc.declare_dram_parameter("out", [128, NTC * 4 * D], FP16, isOutput=True)

    with tile.TileContext(nc) as tc:
        with (
            tc.tile_pool(name="const", bufs=1) as cpool,
            tc.tile_pool(name="work", bufs=4) as wpool,
            tc.tile_pool(name="qk", bufs=2) as qkpool,
            tc.tile_pool(name="ptile", bufs=4) as ppool,
            tc.tile_pool(name="ostage", bufs=2) as opool,
            tc.tile_pool(name="ppb", bufs=2, space="PSUM") as ppb_pool,
            tc.tile_pool(name="pyp", bufs=1, space="PSUM") as pyp_pool,
            tc.tile_pool(name="pd", bufs=1, space="PSUM") as pd_pool,
            tc.tile_pool(name="po", bufs=1, space="PSUM") as po_pool,
        ):
            # ---- persistent SBUF ----
            x_sb = cpool.tile([128, NTC, KC, 512], FP16, tag="x")
            wqkv_sb = cpool.tile([128, KC, 384], FP16, tag="wqkv")
            wpp_sb = cpool.tile([128, D], FP16, tag="wpp")
            cos_sb = cpool.tile([128, TB, 32], FP16, tag="cos")
            sin_sb = cpool.tile([128, TB, 32], FP16, tag="sin")
            diag_sb = cpool.tile([128, 128], FP16, tag="diag")
            ones_sb = cpool.tile([128, 128], FP16, tag="ones")
            qT_sb = cpool.tile([128, T], FP16, tag="qT")  # rows 0:64 g0, 64:128 g1
            kT_sb = cpool.tile([128, T], FP16, tag="kT")
            v_sb = cpool.tile([128, TB, 128], FP16, tag="v")  # [s-part, tb, j]
            ycomb_sb = cpool.tile([128, T], FP16, tag="ycomb")  # [j, t]
            dacc1_sb = cpool.tile([128, 512], FP16, tag="dacc1")
            ssq_sb = cpool.tile([128, TB, 4], F32, tag="ssq")
            bn8_sb = cpool.tile([128, 1], F32, tag="bn8")
            eps_sb = cpool.tile([128, 1], F32, tag="eps")
            c26_sb = cpool.tile([128, 1], F32, tag="c26")
            c15_sb = cpool.tile([128, 1], F32, tag="c15")

            # ---- loads (few, contiguous; first QKV gated on lo-halves only) ----
            nc.sync.dma_start(
                out=wqkv_sb[:, 0:4].rearrange("p a b -> p (a b)"),
                in_=wqkvd[:, 0 : 4 * 384],
            )
            nc.scalar.dma_start(
                out=x_sb[:, 0, 0:4].rearrange("p a b -> p (a b)"),
                in_=xd[:, 0:2048],
            )
            nc.sync.dma_start(
                out=wqkv_sb[:, 4:8].rearrange("p a b -> p (a b)"),
                in_=wqkvd[:, 4 * 384 : 8 * 384],
            )
            nc.scalar.dma_start(
                out=x_sb[:, 0, 4:8].rearrange("p a b -> p (a b)"),
                in_=xd[:, 2048:4096],
            )
            for tc_i in range(1, NTC):
                nc.sync.dma_start(
                    out=x_sb[:, tc_i].rearrange("p a b -> p (a b)"),
                    in_=xd[:, ts(tc_i, KC * 512)],
                )
            nc.gpsimd.dma_start(
                out=cos_sb[:].rearrange("p a b -> p (a b)"), in_=costd[:, 0 : TB * 32]
            )
            nc.gpsimd.dma_start(
                out=sin_sb[:].rearrange("p a b -> p (a b)"),
                in_=costd[:, TB * 32 : 2 * TB * 32],
            )
            nc.gpsimd.dma_start(out=diag_sb[:], in_=diagd[:, :])
            nc.gpsimd.dma_start(out=wpp_sb[:], in_=wppd[:, :])
            nc.vector.memset(ones_sb[:], 1.0)
            nc.vector.memset(bn8_sb[:], -8.0)
            nc.vector.memset(eps_sb[:], EPS)
            nc.vector.memset(c26_sb[:], 2.62)
            nc.vector.memset(c15_sb[:], 1.5)

            # PE warmup during the initial load: ~2.7us of dummy matmuls so
            # HAM reaches K=8/8 before the first real QKV matmul.
            warm = pd_pool.tile([128, 128], F32, tag="pd", name="pd")
            for _ in range(26):
                nc.tensor.matmul(
                    warm[:], ones_sb[:], ones_sb[:], start=True, stop=True
                )

            # ---- stage B: QKV + rmsnorm(Newton) + rotary + batched transpose ----
            def emit_group(G):  # one group of 4 t-blocks
                qkraw = qkpool.tile([128, 4, 256], FP16, tag="qkraw")
                for j in range(4):
                    tb = 4 * G + j
                    pq = ppb_pool.tile([128, 384], F32, tag="ppb")
                    for kc in range(KC):
                        nc.tensor.matmul(
                            pq[:],
                            x_sb[:, G, kc, ts(j, 128)],
                            wqkv_sb[:, kc, :],
                            start=(kc == 0),
                            stop=(kc == KC - 1),
                        )
                    nc.scalar.copy(qkraw[:, j], pq[:, 0:256])
                    nc.scalar.copy(v_sb[:, tb, :], pq[:, 256:384])
                    sq = wpool.tile([128, 256], F32, tag="sq")
                    nc.scalar.square(sq[:], pq[:, 0:256])
                    nc.vector.reduce_sum(
                        ssq_sb[:, tb],
                        sq[:].rearrange("p (h c) -> p h c", c=HD),
                        axis=mybir.AxisListType.X,
                    )

                # Newton rsqrt for the group's 16 (tb, subhead) scales
                epsb = eps_sb[:].unsqueeze(2).broadcast_to([128, 4, 4])
                c26b = c26_sb[:].unsqueeze(2).broadcast_to([128, 4, 4])
                c15b = c15_sb[:].unsqueeze(2).broadcast_to([128, 4, 4])
                m = wpool.tile([128, 4, 4], F32, tag="m")
                nc.vector.scalar_tensor_tensor(
                    m[:], ssq_sb[:, ts(G, 4)], 1.0 / HD, epsb, ALU.mult, ALU.add
                )
                rsc = wpool.tile([128, 4, 4], F32, tag="rsc")
                nc.vector.scalar_tensor_tensor(
                    rsc[:], m[:], -2.22, c26b, ALU.mult, ALU.add
                )
                nc.vector.tensor_scalar(rsc[:], rsc[:], 0.8, 2.5, ALU.max, ALU.min)
                nt = wpool.tile([128, 4, 4], F32, tag="nt")
                for _ in range(3):
                    nc.vector.tensor_mul(nt[:], rsc[:], rsc[:])
                    nc.vector.tensor_mul(nt[:], nt[:], m[:])
                    nc.vector.scalar_tensor_tensor(
                        nt[:], nt[:], -0.5, c15b, ALU.mult, ALU.add
                    )
                    nc.vector.tensor_mul(rsc[:], rsc[:], nt[:])

                # batched rotary over the 4 t-blocks (q and k separately)
                cosb = (
                    cos_sb[:, ts(G, 4), :].unsqueeze(2).broadcast_to([128, 4, 2, 32])
                )
                sinb = (
                    sin_sb[:, ts(G, 4), :].unsqueeze(2).broadcast_to([128, 4, 2, 32])
                )
                stages = []
                for qk in range(2):  # 0: q, 1: k
                    eng = nc.vector
                    raw = qkraw[:, :, ds(qk * 128, 128)].rearrange(
                        "p a (h c) -> p a h c", c=HD
                    )
                    n1 = raw[:, :, :, 0:32]
                    n2 = raw[:, :, :, 32:64]
                    st = wpool.tile([128, 4, 2, HD], FP16, tag=f"st{qk}")
                    tmp = wpool.tile([128, 4, 2, 32], FP16, tag=f"rt1{qk}")
                    tmp2 = wpool.tile([128, 4, 2, 32], FP16, tag=f"rt2{qk}")
                    eng.tensor_mul(tmp[:], n1, cosb)
                    eng.tensor_mul(tmp2[:], n2, sinb)
                    eng.tensor_add(st[:, :, :, 0:32], tmp[:], tmp2[:])
                    eng.tensor_mul(tmp[:], n2, cosb)
                    eng.tensor_mul(tmp2[:], n1, sinb)
                    eng.tensor_sub(st[:, :, :, 32:64], tmp[:], tmp2[:])
                    rscb = (
                        rsc[:, :, ds(qk * 2, 2)]
                        .unsqueeze(3)
                        .broadcast_to([128, 4, 2, HD])
                    )
                    eng.tensor_mul(st[:], st[:], rscb)
                    stages.append(st)

                # batched DMA xbar transposes: [t, 4*128] -> 4 tiles of [c, t]
                nc.sync.dma_start_transpose(
                    qT_sb[:, ts(G, 512)].rearrange("p (a b) -> p a b", a=4),
                    stages[0][:].rearrange("p a h c -> p (a h c)"),
                )
                nc.sync.dma_start_transpose(
                    kT_sb[:, ts(G, 512)].rearrange("p (a b) -> p a b", a=4),
                    stages[1][:].rearrange("p a h c -> p (a h c)"),
                )

            # ---- stage C: differential attention + partial projection ----
            # The projection of chunk tc-1 is software-pipelined into the
            # attention si-loop of chunk tc (one i-half per s-block) so its
            # matmuls and PSUM evacuations fill the exp-wait gaps.
            def emit_proj_half(pc, tb4, half, ost, pool):
                po = pool.tile([128, 512], F32, tag="pd" if pool is pd_pool else "po")
                nc.tensor.matmul(
                    po[:],
                    ycomb_sb[:, ds(pc * 512 + tb4 * 128, 128)],
                    wpp_sb[:, ts(half, 512)],
                    start=True,
                    stop=True,
                )
                if half == 0:
                    nc.vector.tensor_copy(ost[:, tb4, 0:512], po[:])
                else:
                    nc.scalar.copy(ost[:, tb4, 512:1024], po[:])

            def emit_chunk(tc_i):
                nsb = 4 * tc_i + 4  # s-blocks touching this t-chunk
                pyp = pyp_pool.tile([128, 1024], F32, tag="pyp", name="pyp")
                pd0 = pd_pool.tile([128, 512], F32, tag="pd", name="pd")
                if tc_i:
                    ost_prev = opool.tile([128, 4, D], FP16, tag="ost", name="ost")
                else:
                    ost_prev = None
                for si in range(nsb):
                    col0 = max(0, si * 128 - tc_i * 512)
                    w = 512 - col0
                    ppb = ppb_pool.tile([128, 1024], F32, tag="ppb")
                    for g in range(2):
                        nc.tensor.matmul(
                            ppb[:, ds(512 * g + col0, w)],
                            kT_sb[ds(g * 64, 64), ts(si, 128)],
                            qT_sb[ds(g * 64, 64), ds(tc_i * 512 + col0, w)],
                            start=True,
                            stop=True,
                        )
                    pt = ppool.tile([128, 1024], FP16, tag="pt")
                    if col0 == 0:
                        nc.scalar.activation(
                            pt[:], ppb[:], AF.Exp, bias=bn8_sb[:], scale=SCALE
                        )
                    else:
                        # one strided call covering both groups' live columns
                        esrc = ppb[:].rearrange("p (g t) -> p g t", g=2)[
                            :, :, ds(col0, w)
                        ]
                        edst = pt[:].rearrange("p (g t) -> p g t", g=2)[
                            :, :, ds(col0, w)
                        ]
                        nc.scalar.activation(
                            edst, esrc, AF.Exp, bias=bn8_sb[:], scale=SCALE
                        )
                    if col0 > 0 or si * 128 == tc_i * 512:
                        # diagonal block: zero out s > t inside it (both groups)
                        dslc = (
                            pt[:]
                            .rearrange("p (g t) -> p g t", g=2)[:, :, ds(col0, 128)]
                        )
                        nc.vector.tensor_mul(
                            dslc,
                            dslc,
                            diag_sb[:].unsqueeze(1).broadcast_to([128, 2, 128]),
                        )
                    last = si == nsb - 1
                    for g in range(2):
                        nc.tensor.matmul(
                            pyp[:, ds(512 * g + col0, w)],
                            v_sb[:, si, :],
                            pt[:, ds(512 * g + col0, w)],
                            start=(si == 0),
                            stop=last,
                        )
                    # denom g0 on PE
                    nc.tensor.matmul(
                        pd0[:, ds(col0, w)],
                        ones_sb[:],
                        pt[:, ds(col0, w)],
                        start=(si == 0),
                        stop=last,
                    )
                    # denom g1 accumulates on gpsimd (off the critical path)
                    if si == 0:
                        nc.vector.tensor_copy(dacc1_sb[:], pt[:, 512:1024])
                    else:
                        nc.gpsimd.tensor_add(
                            dacc1_sb[:, ds(col0, w)],
                            dacc1_sb[:, ds(col0, w)],
                            pt[:, ds(512 + col0, w)],
                        )
                    # pipelined projection of the previous chunk
                    if tc_i and si < 8:
                        emit_proj_half(tc_i - 1, si // 2, si % 2, ost_prev, po_pool)
                        if si == 7:
                            nc.sync.dma_start(
                                out=outd[:, ts(tc_i - 1, 4 * D)],
                                in_=ost_prev[:].rearrange("p a b -> p (a b)"),
                            )

                rec0 = wpool.tile([128, 512], F32, tag="rec0")
                nc.vector.reciprocal_approx_fast(rec0[:], pd0[:])
                pdb1 = po_pool.tile([128, 512], F32, tag="po", name="po")
                nc.tensor.matmul(
                    pdb1[:], ones_sb[:], dacc1_sb[:], start=True, stop=True
                )
                rec1 = wpool.tile([128, 512], F32, tag="rec1")
                nc.vector.reciprocal_approx_fast(rec1[:], pdb1[:])
                yn0 = wpool.tile([128, 512], F32, tag="yn0")
                nc.vector.tensor_mul(yn0[:], pyp[:, 0:512], rec0[:])
                yn1 = wpool.tile([128, 512], F32, tag="yn1")
                nc.vector.tensor_mul(yn1[:], pyp[:, 512:1024], rec1[:])
                nc.vector.scalar_tensor_tensor(
                    ycomb_sb[:, ts(tc_i, 512)],
                    yn1[:],
                    -lam,
                    yn0[:],
                    ALU.mult,
                    ALU.add,
                )

            # interleave stage B groups with stage C chunks (staggered by one)
            # so attention's ACT/DVE load overlaps the projection's PE load
            emit_group(0)
            emit_group(1)
            emit_chunk(0)
            emit_group(2)
            emit_chunk(1)
            emit_group(3)
            emit_chunk(2)
            emit_chunk(3)

            # final chunk's projection (alternating the two free PSUM banks)
            ost = opool.tile([128, 4, D], FP16, tag="ost")
            for tb4 in range(4):
                for half in range(2):
                    pool = po_pool if (tb4 * 2 + half) % 2 == 0 else pd_pool
                    emit_proj_half(NTC - 1, tb4, half, ost, pool)
                nc.sync.dma_start(
                    out=outd[
                        :, (NTC - 1) * 4 * D + tb4 * D : (NTC - 1) * 4 * D + (tb4 + 1) * D
                    ],
                    in_=ost[:, tb4].rearrange("p b -> p b"),
                )

    nc.compile()
    return nc


def _make_in_maps(x, Wq, Wk, Wv, Wproj):
    f16 = np.float16
    xT = np.ascontiguousarray(x[0].T).astype(f16)  # [D, T]
    # x repack: [p, tc, kc, u] = x[0, tc*512+u, kc*128+p]
    xr = xT.reshape(KC, 128, NTC, 512).transpose(1, 2, 0, 3).reshape(128, -1)
    xr = np.ascontiguousarray(xr)

    # rotary tables, [tp, tb, 32] layout, cos|sin concatenated
    inv = 1.0 / (10000.0 ** (np.arange(0, HD, 2, dtype=np.float32) / HD))
    fr = np.outer(np.arange(T, dtype=np.float32), inv)  # [T, 32]
    cos = np.cos(fr).reshape(TB, 128, 32).transpose(1, 0, 2).reshape(128, -1)
    sin = np.sin(fr).reshape(TB, 128, 32).transpose(1, 0, 2).reshape(128, -1)
    cost = np.concatenate([cos, sin], axis=1).astype(f16)
    diag = np.triu(np.ones((128, 128), np.float32)).astype(f16)

    in_maps = []
    for h in range(NCORES):
        wcat = np.concatenate(
            [
                Wq[h * 64 : h * 64 + 64],
                Wq[512 + h * 64 : 512 + h * 64 + 64],
                Wk[h * 64 : h * 64 + 64],
                Wk[512 + h * 64 : 512 + h * 64 + 64],
                Wv[h * 128 : h * 128 + 128],
            ],
            axis=0,
        ).T  # [D, 384]
        wqkv = wcat.reshape(KC, 128, 384).transpose(1, 0, 2).reshape(128, -1)
        wpp = Wproj[:, h * 128 : (h + 1) * 128].T  # [j, i]
        in_maps.append(
            {
                "x": xr,
                "wqkv": np.ascontiguousarray(wqkv).astype(f16),
                "wpp": np.ascontiguousarray(wpp).astype(f16),
                "cost": cost,
                "diag": diag,
            }
        )
    return in_maps


def _get_program(lam: float):
    key = round(lam, 10)
    if key not in _CACHE:
        _CACHE[key] = _build_program(lam)
    return _CACHE[key]


def kernel(x, Wq, Wk, Wv, Wproj, lambda_q1, lambda_k1, lambda_q2, lambda_k2):
    x = np.asarray(x, np.float32)
    Wq, Wk = np.asarray(Wq, np.float32), np.asarray(Wk, np.float32)
    Wv, Wproj = np.asarray(Wv, np.float32), np.asarray(Wproj, np.float32)

    lam1 = float(np.exp(np.sum(np.asarray(lambda_q1) * np.asarray(lambda_k1))))
    lam2 = float(np.exp(np.sum(np.asarray(lambda_q2) * np.asarray(lambda_k2))))
    lam = lam1 - lam2 + LAMBDA_INIT

    in_maps = _make_in_maps(x, Wq, Wk, Wv, Wproj)
    nc = _get_program(lam)

    res = run_bass_kernel_spmd(nc, in_maps, list(range(NCORES)))
    acc = res.results[0]["out"].astype(np.float32)
    for h in range(1, NCORES):
        acc += res.results[h]["out"].astype(np.float32)
    y = acc.reshape(128, NTC, 4, D).transpose(1, 2, 0, 3).reshape(1, T, D)
    return np.ascontiguousarray(y)


if __name__ == "__main__":
    rng = np.random.default_rng(0)
    ins = {
        "x": rng.standard_normal((1, T, D), np.float32),
        "Wq": (rng.standard_normal((D, D)) * 0.02).astype(np.float32),
        "Wk": (rng.standard_normal((D, D)) * 0.02).astype(np.float32),
        "Wv": (rng.standard_normal((D, D)) * 0.02).astype(np.float32),
        "Wproj": (rng.standard_normal((D, D)) * 0.02).astype(np.float32),
        "lambda_q1": (rng.standard_normal(32) * 0.1).astype(np.float32),
        "lambda_k1": (rng.standard_normal(32) * 0.1).astype(np.float32),
        "lambda_q2": (rng.standard_normal(32) * 0.1).astype(np.float32),
        "lambda_k2": (rng.standard_normal(32) * 0.1).astype(np.float32),
    }
    y = kernel(**ins)
    print("kernel output", y.shape, y.dtype, float(np.abs(y).mean()))


# revision 37
# speedup vs baseline: 1.0369x; 1.0369x over previous
"""Trainium2 Bass kernel for MixerDiffAttention (differential attention).

Sharding: tensor-parallel over the 8 (n_head//2) head groups across 8 cores.
Each core computes QKV projections for its head group, both differential
attention branches, the normalized combination y1 - lambda*y2, and its head's
partial product with the row-sharded c_proj. The host sums the 8 partial
outputs.

v3 notes:
  - All tensors fp16 on-chip; every DMA is per-partition contiguous.
  - Stage B works in groups of 4 t-blocks: QKV matmuls -> ACT stages raw q/k
    to SBUF -> DVE computes the rms scale with a Newton rsqrt (linear init,
    clipped, 3 iterations) so the only ACT table set ever loaded is exp's ->
    batched rotary -> one batched DMA xbar transpose per group for qT/kT.
  - scores exp'ed with bias -8 (|score*scale| <= 8 after rms norm) into fp16.
  - softmax denominators: group 0 accumulates on PE (ones-matmul per s-block),
    group 1 on DVE in fp16 (2x packed); one ones-matmul per t-chunk reduces
    group 1 across partitions.
  - partial projection uses ycomb chunks as the stationary operand, giving
    [t, i]-layout outputs; PSUM evacuation alternates DVE/ACT; projection
    PSUM is double-buffered via the pd pool ring.
"""

import os
import sys

import numpy as np

for _p in ("/opt/trn_rl_repo", "/root/.axon_site/_ro/trn_rl_repo"):
    if os.path.isdir(_p) and _p not in sys.path:
        sys.path.insert(0, _p)

import concourse.bass as bass
import concourse.mybir as mybir
import concourse.tile as tile
from concourse import bacc
from concourse.bass import ds, ts
from concourse.bass_utils import run_bass_kernel_spmd

FP16 = mybir.dt.float16
F32 = mybir.dt.float32
AF = mybir.ActivationFunctionType
ALU = mybir.AluOpType

N_HEAD = 16
D = 1024
HD = 64  # head dim
T = 2048
NCORES = 8
TB = T // 128  # 16 t-blocks
KC = D // 128  # 8 contraction chunks
NTC = T // 512  # 4 t-chunks of 512
LAMBDA_INIT = 0.8 - 0.6 * float(np.exp(-0.3 * 1))
EPS = float(np.finfo(np.float32).eps)
SCALE = 1.0 / 8.0  # 1/sqrt(64)

_CACHE = {}


def _build_program(lam: float) -> bass.Bass:
    nc = bacc.Bacc("TRN2", target_bir_lowering=False, debug=False)

    xd = nc.declare_dram_parameter("x", [128, NTC * KC * 512], FP16, isOutput=False)
    wqkvd = nc.declare_dram_parameter("wqkv", [128, KC * 384], FP16, isOutput=False)
    wppd = nc.declare_dram_parameter("wpp", [128, D], FP16, isOutput=False)
    costd = nc.declare_dram_parameter("cost", [128, 2 * TB * 32], FP16, isOutput=False)
    diagd = nc.declare_dram_parameter("diag", [128, 128], FP16, isOutput=False)
    outd = nc.declare_dram_parameter("out", [128, NTC * 4 * D], FP16, isOutput=True)

    with tile.TileContext(nc) as tc:
        with (
            tc.tile_pool(name="const", bufs=1) as cpool,
            tc.tile_pool(name="work", bufs=4) as wpool,
            tc.tile_pool(name="qk", bufs=2) as qkpool,
            tc.tile_pool(name="ptile", bufs=4) as ppool,
            tc.tile_pool(name="ostage", bufs=2) as opool,
            tc.tile_pool(name="ppb", bufs=2, space="PSUM") as ppb_pool,
            tc.tile_pool(name="pyp", bufs=1, space="PSUM") as pyp_pool,
            tc.tile_pool(name="pd", bufs=1, space="PSUM") as pd_pool,
            tc.tile_pool(name="po", bufs=1, space="PSUM") as po_pool,
        ):
            # ---- persistent SBUF ----
            x_sb = cpool.tile([128, NTC, KC, 512], FP16, tag="x")
            wqkv_sb = cpool.tile([128, KC, 384], FP16, tag="wqkv")
            wpp_sb = cpool.tile([128, D], FP16, tag="wpp")
            cos_sb = cpool.tile([128, TB, 32], FP16, tag="cos")
            sin_sb = cpool.tile([128, TB, 32], FP16, tag="sin")
            diag_sb = cpool.tile([128, 128], FP16, tag="diag")
            ones_sb = cpool.tile([128, 128], FP16, tag="ones")
            qT_sb = cpool.tile([128, T], FP16, tag="qT")  # rows 0:64 g0, 64:128 g1
            kT_sb = cpool.tile([128, T], FP16, tag="kT")
            v_sb = cpool.tile([128, TB, 128], FP16, tag="v")  # [s-part, tb, j]
            ycomb_sb = cpool.tile([128, T], FP16, tag="ycomb")  # [j, t]
            dacc1_sb = cpool.tile([128, 512], FP16, tag="dacc1")
            ssq_sb = cpool.tile([128, TB, 4], F32, tag="ssq")
            bn8_sb = cpool.tile([128, 1], F32, tag="bn8")
            eps_sb = cpool.tile([128, 1], F32, tag="eps")
            c26_sb = cpool.tile([128, 1], F32, tag="c26")
            c15_sb = cpool.tile([128, 1], F32, tag="c15")

            # ---- loads (few, contiguous; first QKV gated on lo-halves only) ----
            nc.sync.dma_start(
                out=wqkv_sb[:, 0:4].rearrange("p a b -> p (a b)"),
                in_=wqkvd[:, 0 : 4 * 384],
            )
            nc.scalar.dma_start(
                out=x_sb[:, 0, 0:4].rearrange("p a b -> p (a b)"),
                in_=xd[:, 0:2048],
            )
            nc.sync.dma_start(
                out=wqkv_sb[:, 4:8].rearrange("p a b -> p (a b)"),
                in_=wqkvd[:, 4 * 384 : 8 * 384],
            )
            nc.scalar.dma_start(
                out=x_sb[:, 0, 4:8].rearrange("p a b -> p (a b)"),
                in_=xd[:, 2048:4096],
            )
            for tc_i in range(1, NTC):
                nc.sync.dma_start(
                    out=x_sb[:, tc_i].rearrange("p a b -> p (a b)"),
                    in_=xd[:, ts(tc_i, KC * 512)],
                )
            nc.gpsimd.dma_start(
                out=cos_sb[:].rearrange("p a b -> p (a b)"), in_=costd[:, 0 : TB * 32]
            )
            nc.gpsimd.dma_start(
                out=sin_sb[:].rearrange("p a b -> p (a b)"),
                in_=costd[:, TB * 32 : 2 * TB * 32],
            )
            nc.gpsimd.dma_start(out=diag_sb[:], in_=diagd[:, :])
            nc.gpsimd.dma_start(out=wpp_sb[:], in_=wppd[:, :])
            nc.vector.memset(ones_sb[:], 1.0)
            nc.vector.memset(bn8_sb[:], -8.0)
            nc.vector.memset(eps_sb[:], EPS)
            nc.vector.memset(c26_sb[:], 2.62)
            nc.vector.memset(c15_sb[:], 1.5)

            # PE warmup during the initial load: ~2.7us of dummy matmuls so
            # HAM reaches K=8/8 before the first real QKV matmul.
            warm = pd_pool.tile([128, 128], F32, tag="pd", name="pd")
            for _ in range(26):
                nc.tensor.matmul(
                    warm[:], ones_sb[:], ones_sb[:], start=True, stop=True
                )

            # ---- stage B: QKV + rmsnorm(Newton) + rotary + batched transpose ----
            def emit_group(G):  # one group of 4 t-blocks
                qkraw = qkpool.tile([128, 4, 256], FP16, tag="qkraw")
                for j in range(4):
                    tb = 4 * G + j
                    pq = ppb_pool.tile([128, 384], F32, tag="ppb")
                    for kc in range(KC):
                        nc.tensor.matmul(
                            pq[:],
                            x_sb[:, G, kc, ts(j, 128)],
                            wqkv_sb[:, kc, :],
                            start=(kc == 0),
                            stop=(kc == KC - 1),
                        )
                    nc.scalar.copy(qkraw[:, j], pq[:, 0:256])
                    nc.scalar.copy(v_sb[:, tb, :], pq[:, 256:384])
                    sq = wpool.tile([128, 256], F32, tag="sq")
                    nc.scalar.square(sq[:], pq[:, 0:256])
                    nc.vector.reduce_sum(
                        ssq_sb[:, tb],
                        sq[:].rearrange("p (h c) -> p h c", c=HD),
                        axis=mybir.AxisListType.X,
                    )

                # Newton rsqrt for the group's 16 (tb, subhead) scales
                epsb = eps_sb[:].unsqueeze(2).broadcast_to([128, 4, 4])
                c26b = c26_sb[:].unsqueeze(2).broadcast_to([128, 4, 4])
                c15b = c15_sb[:].unsqueeze(2).broadcast_to([128, 4, 4])
                m = wpool.tile([128, 4, 4], F32, tag="m")
                nc.vector.scalar_tensor_tensor(
                    m[:], ssq_sb[:, ts(G, 4)], 1.0 / HD, epsb, ALU.mult, ALU.add
                )
                rsc = wpool.tile([128, 4, 4], F32, tag="rsc")
                nc.vector.scalar_tensor_tensor(
                    rsc[:], m[:], -2.22, c26b, ALU.mult, ALU.add
                )
                nc.vector.tensor_scalar(rsc[:], rsc[:], 0.8, 2.5, ALU.max, ALU.min)
                nt = wpool.tile([128, 4, 4], F32, tag="nt")
                for _ in range(3):
                    nc.vector.tensor_mul(nt[:], rsc[:], rsc[:])
                    nc.vector.tensor_mul(nt[:], nt[:], m[:])
                    nc.vector.scalar_tensor_tensor(
                        nt[:], nt[:], -0.5, c15b, ALU.mult, ALU.add
                    )
                    nc.vector.tensor_mul(rsc[:], rsc[:], nt[:])

                # batched rotary over the 4 t-blocks (q and k separately)
                cosb = (
                    cos_sb[:, ts(G, 4), :].unsqueeze(2).broadcast_to([128, 4, 2, 32])
                )
                sinb = (
                    sin_sb[:, ts(G, 4), :].unsqueeze(2).broadcast_to([128, 4, 2, 32])
                )
                stages = []
                for qk in range(2):  # 0: q, 1: k
                    eng = nc.vector
                    raw = qkraw[:, :, ds(qk * 128, 128)].rearrange(
                        "p a (h c) -> p a h c", c=HD
                    )
                    n1 = raw[:, :, :, 0:32]
                    n2 = raw[:, :, :, 32:64]
                    st = wpool.tile([128, 4, 2, HD], FP16, tag=f"st{qk}")
                    tmp = wpool.tile([128, 4, 2, 32], FP16, tag=f"rt1{qk}")
                    tmp2 = wpool.tile([128, 4, 2, 32], FP16, tag=f"rt2{qk}")
                    eng.tensor_mul(tmp[:], n1, cosb)
                    eng.tensor_mul(tmp2[:], n2, sinb)
                    eng.tensor_add(st[:, :, :, 0:32], tmp[:], tmp2[:])
                    eng.tensor_mul(tmp[:], n2, cosb)
                    eng.tensor_mul(tmp2[:], n1, sinb)
                    eng.tensor_sub(st[:, :, :, 32:64], tmp[:], tmp2[:])
                    rscb = (
                        rsc[:, :, ds(qk * 2, 2)]
                        .unsqueeze(3)
                        .broadcast_to([128, 4, 2, HD])
                    )
                    eng.tensor_mul(st[:], st[:], rscb)
                    stages.append(st)

                # batched DMA xbar transposes: [t, 4*128] -> 4 tiles of [c, t]
                nc.sync.dma_start_transpose(
                    qT_sb[:, ts(G, 512)].rearrange("p (a b) -> p a b", a=4),
                    stages[0][:].rearrange("p a h c -> p (a h c)"),
                )
                nc.sync.dma_start_transpose(
                    kT_sb[:, ts(G, 512)].rearrange("p (a b) -> p a b", a=4),
                    stages[1][:].rearrange("p a h c -> p (a h c)"),
                )

            # ---- stage C: differential attention + partial projection ----
            # The projection of chunk tc-1 is software-pipelined into the
            # attention si-loop of chunk tc (one i-half per s-block) so its
            # matmuls and PSUM evacuations fill the exp-wait gaps.
            def emit_proj_half(pc, tb4, half, ost, pool):
                po = pool.tile([128, 512], F32, tag="pd" if pool is pd_pool else "po")
                nc.tensor.matmul(
                    po[:],
                    ycomb_sb[:, ds(pc * 512 + tb4 * 128, 128)],
                    wpp_sb[:, ts(half, 512)],
                    start=True,
                    stop=True,
                )
                if half == 0:
                    nc.vector.tensor_copy(ost[:, tb4, 0:512], po[:])
                else:
                    nc.scalar.copy(ost[:, tb4, 512:1024], po[:])

            def emit_chunk(tc_i):
                nsb = 4 * tc_i + 4  # s-blocks touching this t-chunk
                pyp = pyp_pool.tile([128, 1024], F32, tag="pyp", name="pyp")
                pd0 = pd_pool.tile([128, 512], F32, tag="pd", name="pd")
                if tc_i:
                    ost_prev = opool.tile([128, 4, D], FP16, tag="ost", name="ost")
                else:
                    ost_prev = None
                for si in range(nsb):
                    col0 = max(0, si * 128 - tc_i * 512)
                    w = 512 - col0
                    ppb = ppb_pool.tile([128, 1024], F32, tag="ppb")
                    for g in range(2):
                        nc.tensor.matmul(
                            ppb[:, ds(512 * g + col0, w)],
                            kT_sb[ds(g * 64, 64), ts(si, 128)],
                            qT_sb[ds(g * 64, 64), ds(tc_i * 512 + col0, w)],
                            start=True,
                            stop=True,
                        )
                    pt = ppool.tile([128, 1024], FP16, tag="pt")
                    if col0 == 0:
                        nc.scalar.activation(
                            pt[:], ppb[:], AF.Exp, bias=bn8_sb[:], scale=SCALE
                        )
                    else:
                        # one strided call covering both groups' live columns
                        esrc = ppb[:].rearrange("p (g t) -> p g t", g=2)[
                            :, :, ds(col0, w)
                        ]
                        edst = pt[:].rearrange("p (g t) -> p g t", g=2)[
                            :, :, ds(col0, w)
                        ]
                        nc.scalar.activation(
                            edst, esrc, AF.Exp, bias=bn8_sb[:], scale=SCALE
                        )
                    if col0 > 0 or si * 128 == tc_i * 512:
                        # diagonal block: zero out s > t inside it (both groups)
                        dslc = (
                            pt[:]
                            .rearrange("p (g t) -> p g t", g=2)[:, :, ds(col0, 128)]
                        )
                        nc.vector.tensor_mul(
                            dslc,
                            dslc,
                            diag_sb[:].unsqueeze(1).broadcast_to([128, 2, 128]),
                        )
                    last = si == nsb - 1
                    for g in range(2):
                        nc.tensor.matmul(
                            pyp[:, ds(512 * g + col0, w)],
                            v_sb[:, si, :],
                            pt[:, ds(512 * g + col0, w)],
                            start=(si == 0),
                            stop=last,
                        )
                    # denom g0 on PE
                    nc.tensor.matmul(
                        pd0[:, ds(col0, w)],
                        ones_sb[:],
                        pt[:, ds(col0, w)],
                        start=(si == 0),
                        stop=last,
                    )
                    # denom g1 accumulates on gpsimd (off the critical path)
                    if si == 0:
                        nc.vector.tensor_copy(dacc1_sb[:], pt[:, 512:1024])
                    else:
                        nc.gpsimd.tensor_add(
                            dacc1_sb[:, ds(col0, w)],
                            dacc1_sb[:, ds(col0, w)],
                            pt[:, ds(512 + col0, w)],
                        )
                    # pipelined projection of the previous chunk
                    if tc_i and si < 8:
                        emit_proj_half(tc_i - 1, si // 2, si % 2, ost_prev, po_pool)
                        if si == 7:
                            nc.sync.dma_start(
                                out=outd[:, ts(tc_i - 1, 4 * D)],
                                in_=ost_prev[:].rearrange("p a b -> p (a b)"),
                            )

                rec0 = wpool.tile([128, 512], F32, tag="rec0")
                nc.vector.reciprocal_approx_fast(rec0[:], pd0[:])
                pdb1 = po_pool.tile([128, 512], F32, tag="po", name="po")
                nc.tensor.matmul(
                    pdb1[:], ones_sb[:], dacc1_sb[:], start=True, stop=True
                )
                rec1 = wpool.tile([128, 512], F32, tag="rec1")
                nc.vector.reciprocal_approx_fast(rec1[:], pdb1[:])
                yn0 = wpool.tile([128, 512], F32, tag="yn0")
                nc.vector.tensor_mul(yn0[:], pyp[:, 0:512], rec0[:])
                yn1 = wpool.tile([128, 512], F32, tag="yn1")
                nc.vector.tensor_mul(yn1[:], pyp[:, 512:1024], rec1[:])
                nc.vector.scalar_tensor_tensor(
                    ycomb_sb[:, ts(tc_i, 512)],
                    yn1[:],
                    -lam,
                    yn0[:],
                    ALU.mult,
                    ALU.add,
                )

            for G in range(4):
                emit_group(G)
            for tc_i in range(NTC):
                emit_chunk(tc_i)

            # final chunk's projection (alternating the two free PSUM banks)
            ost = opool.tile([128, 4, D], FP16, tag="ost")
            for tb4 in range(4):
                for half in range(2):
                    pool = po_pool if (tb4 * 2 + half) % 2 == 0 else pd_pool
                    emit_proj_half(NTC - 1, tb4, half, ost, pool)
                nc.sync.dma_start(
                    out=outd[
                        :, (NTC - 1) * 4 * D + tb4 * D : (NTC - 1) * 4 * D + (tb4 + 1) * D
                    ],
                    in_=ost[:, tb4].rearrange("p b -> p b"),
                )

    nc.compile()
    return nc


def _make_in_maps(x, Wq, Wk, Wv, Wproj):
    f16 = np.float16
    xT = np.ascontiguousarray(x[0].T).astype(f16)  # [D, T]
    # x repack: [p, tc, kc, u] = x[0, tc*512+u, kc*128+p]
    xr = xT.reshape(KC, 128, NTC, 512).transpose(1, 2, 0, 3).reshape(128, -1)
    xr = np.ascontiguousarray(xr)

    # rotary tables, [tp, tb, 32] layout, cos|sin concatenated
    inv = 1.0 / (10000.0 ** (np.arange(0, HD, 2, dtype=np.float32) / HD))
    fr = np.outer(np.arange(T, dtype=np.float32), inv)  # [T, 32]
    cos = np.cos(fr).reshape(TB, 128, 32).transpose(1, 0, 2).reshape(128, -1)
    sin = np.sin(fr).reshape(TB, 128, 32).transpose(1, 0, 2).reshape(128, -1)
    cost = np.concatenate([cos, sin], axis=1).astype(f16)
    diag = np.triu(np.ones((128, 128), np.float32)).astype(f16)

    in_maps = []
    for h in range(NCORES):
        wcat = np.concatenate(
            [
                Wq[h * 64 : h * 64 + 64],
                Wq[512 + h * 64 : 512 + h * 64 + 64],
                Wk[h * 64 : h * 64 + 64],
                Wk[512 + h * 64 : 512 + h * 64 + 64],
                Wv[h * 128 : h * 128 + 128],
            ],
            axis=0,
        ).T  # [D, 384]
        wqkv = wcat.reshape(KC, 128, 384).transpose(1, 0, 2).reshape(128, -1)
        wpp = Wproj[:, h * 128 : (h + 1) * 128].T  # [j, i]
        in_maps.append(
            {
                "x": xr,
                "wqkv": np.ascontiguousarray(wqkv).astype(f16),
                "wpp": np.ascontiguousarray(wpp).astype(f16),
                "cost": cost,
                "diag": diag,
            }
        )
    return in_maps


def _get_program(lam: float):
    key = round(lam, 10)
    if key not in _CACHE:
        _CACHE[key] = _build_program(lam)
    return _CACHE[key]


def kernel(x, Wq, Wk, Wv, Wproj, lambda_q1, lambda_k1, lambda_q2, lambda_k2):
    x = np.asarray(x, np.float32)
    Wq, Wk = np.asarray(Wq, np.float32), np.asarray(Wk, np.float32)
    Wv, Wproj = np.asarray(Wv, np.float32), np.asarray(Wproj, np.float32)

    lam1 = float(np.exp(np.sum(np.asarray(lambda_q1) * np.asarray(lambda_k1))))
    lam2 = float(np.exp(np.sum(np.asarray(lambda_q2) * np.asarray(lambda_k2))))
    lam = lam1 - lam2 + LAMBDA_INIT

    in_maps = _make_in_maps(x, Wq, Wk, Wv, Wproj)
    nc = _get_program(lam)

    res = run_bass_kernel_spmd(nc, in_maps, list(range(NCORES)))
    acc = res.results[0]["out"].astype(np.float32)
    for h in range(1, NCORES):
        acc += res.results[h]["out"].astype(np.float32)
    y = acc.reshape(128, NTC, 4, D).transpose(1, 2, 0, 3).reshape(1, T, D)
    return np.ascontiguousarray(y)


if __name__ == "__main__":
    rng = np.random.default_rng(0)
    ins = {
        "x": rng.standard_normal((1, T, D), np.float32),
        "Wq": (rng.standard_normal((D, D)) * 0.02).astype(np.float32),
        "Wk": (rng.standard_normal((D, D)) * 0.02).astype(np.float32),
        "Wv": (rng.standard_normal((D, D)) * 0.02).astype(np.float32),
        "Wproj": (rng.standard_normal((D, D)) * 0.02).astype(np.float32),
        "lambda_q1": (rng.standard_normal(32) * 0.1).astype(np.float32),
        "lambda_k1": (rng.standard_normal(32) * 0.1).astype(np.float32),
        "lambda_q2": (rng.standard_normal(32) * 0.1).astype(np.float32),
        "lambda_k2": (rng.standard_normal(32) * 0.1).astype(np.float32),
    }
    y = kernel(**ins)
    print("kernel output", y.shape, y.dtype, float(np.abs(y).mean()))


# revision 38
# speedup vs baseline: 1.0478x; 1.0105x over previous
"""Trainium2 Bass kernel for MixerDiffAttention (differential attention).

Sharding: tensor-parallel over the 8 (n_head//2) head groups across 8 cores.
Each core computes QKV projections for its head group, both differential
attention branches, the normalized combination y1 - lambda*y2, and its head's
partial product with the row-sharded c_proj. The host sums the 8 partial
outputs.

v3 notes:
  - All tensors fp16 on-chip; every DMA is per-partition contiguous.
  - Stage B works in groups of 4 t-blocks: QKV matmuls -> ACT stages raw q/k
    to SBUF -> DVE computes the rms scale with a Newton rsqrt (linear init,
    clipped, 3 iterations) so the only ACT table set ever loaded is exp's ->
    batched rotary -> one batched DMA xbar transpose per group for qT/kT.
  - scores exp'ed with bias -8 (|score*scale| <= 8 after rms norm) into fp16.
  - softmax denominators: group 0 accumulates on PE (ones-matmul per s-block),
    group 1 on DVE in fp16 (2x packed); one ones-matmul per t-chunk reduces
    group 1 across partitions.
  - partial projection uses ycomb chunks as the stationary operand, giving
    [t, i]-layout outputs; PSUM evacuation alternates DVE/ACT; projection
    PSUM is double-buffered via the pd pool ring.
"""

import os
import sys

import numpy as np

for _p in ("/opt/trn_rl_repo", "/root/.axon_site/_ro/trn_rl_repo"):
    if os.path.isdir(_p) and _p not in sys.path:
        sys.path.insert(0, _p)

import concourse.bass as bass
import concourse.mybir as mybir
import concourse.tile as tile
from concourse import bacc
from concourse.bass import ds, ts
from concourse.bass_utils import run_bass_kernel_spmd

FP16 = mybir.dt.float16
F32 = mybir.dt.float32
AF = mybir.ActivationFunctionType
ALU = mybir.AluOpType

N_HEAD = 16
D = 1024
HD = 64  # head dim
T = 2048
NCORES = 8
TB = T // 128  # 16 t-blocks
KC = D // 128  # 8 contraction chunks
NTC = T // 512  # 4 t-chunks of 512
LAMBDA_INIT = 0.8 - 0.6 * float(np.exp(-0.3 * 1))
EPS = float(np.finfo(np.float32).eps)
SCALE = 1.0 / 8.0  # 1/sqrt(64)

_CACHE = {}


def _build_program(lam: float) -> bass.Bass:
    nc = bacc.Bacc("TRN2", target_bir_lowering=False, debug=False)

    xd = nc.declare_dram_parameter("x", [128, NTC * KC * 512], FP16, isOutput=False)
    wqkvd = nc.declare_dram_parameter("wqkv", [128, KC * 384], FP16, isOutput=False)
    wppd = nc.declare_dram_parameter("wpp", [128, D], FP16, isOutput=False)
    costd = nc.declare_dram_parameter("cost", [128, 2 * TB * 32], FP16, isOutput=False)
    diagd = nc.declare_dram_parameter("diag", [128, 128], FP16, isOutput=False)
    outd = nc.declare_dram_parameter("out", [128, NTC * 4 * D], FP16, isOutput=True)

    with tile.TileContext(nc) as tc:
        with (
            tc.tile_pool(name="const", bufs=1) as cpool,
            tc.tile_pool(name="work", bufs=4) as wpool,
            tc.tile_pool(name="qk", bufs=2) as qkpool,
            tc.tile_pool(name="ptile", bufs=4) as ppool,
            tc.tile_pool(name="ostage", bufs=2) as opool,
            tc.tile_pool(name="ppb", bufs=2, space="PSUM") as ppb_pool,
            tc.tile_pool(name="pyp", bufs=1, space="PSUM") as pyp_pool,
            tc.tile_pool(name="pd", bufs=1, space="PSUM") as pd_pool,
            tc.tile_pool(name="po", bufs=1, space="PSUM") as po_pool,
        ):
            # ---- persistent SBUF ----
            x_sb = cpool.tile([128, NTC, KC, 512], FP16, tag="x")
            wqkv_sb = cpool.tile([128, KC, 384], FP16, tag="wqkv")
            wpp_sb = cpool.tile([128, D], FP16, tag="wpp")
            cos_sb = cpool.tile([128, TB, 32], FP16, tag="cos")
            sin_sb = cpool.tile([128, TB, 32], FP16, tag="sin")
            diag_sb = cpool.tile([128, 128], FP16, tag="diag")
            ones_sb = cpool.tile([128, 128], FP16, tag="ones")
            qT_sb = cpool.tile([128, T], FP16, tag="qT")  # rows 0:64 g0, 64:128 g1
            kT_sb = cpool.tile([128, T], FP16, tag="kT")
            v_sb = cpool.tile([128, TB, 128], FP16, tag="v")  # [s-part, tb, j]
            ycomb_sb = cpool.tile([128, T], FP16, tag="ycomb")  # [j, t]
            dacc1_sb = cpool.tile([128, 512], FP16, tag="dacc1")
            ssq_sb = cpool.tile([128, TB, 4], F32, tag="ssq")
            bn8_sb = cpool.tile([128, 1], F32, tag="bn8")
            eps_sb = cpool.tile([128, 1], F32, tag="eps")
            c26_sb = cpool.tile([128, 1], F32, tag="c26")
            c15_sb = cpool.tile([128, 1], F32, tag="c15")

            # ---- loads (few, contiguous; first QKV gated on lo-halves only) ----
            nc.sync.dma_start(
                out=wqkv_sb[:, 0:4].rearrange("p a b -> p (a b)"),
                in_=wqkvd[:, 0 : 4 * 384],
            )
            nc.scalar.dma_start(
                out=x_sb[:, 0, 0:4].rearrange("p a b -> p (a b)"),
                in_=xd[:, 0:2048],
            )
            nc.sync.dma_start(
                out=wqkv_sb[:, 4:8].rearrange("p a b -> p (a b)"),
                in_=wqkvd[:, 4 * 384 : 8 * 384],
            )
            nc.scalar.dma_start(
                out=x_sb[:, 0, 4:8].rearrange("p a b -> p (a b)"),
                in_=xd[:, 2048:4096],
            )
            for tc_i in range(1, NTC):
                nc.sync.dma_start(
                    out=x_sb[:, tc_i].rearrange("p a b -> p (a b)"),
                    in_=xd[:, ts(tc_i, KC * 512)],
                )
            nc.gpsimd.dma_start(
                out=cos_sb[:].rearrange("p a b -> p (a b)"), in_=costd[:, 0 : TB * 32]
            )
            nc.gpsimd.dma_start(
                out=sin_sb[:].rearrange("p a b -> p (a b)"),
                in_=costd[:, TB * 32 : 2 * TB * 32],
            )
            nc.gpsimd.dma_start(out=diag_sb[:], in_=diagd[:, :])
            nc.gpsimd.dma_start(out=wpp_sb[:], in_=wppd[:, :])
            nc.vector.memset(ones_sb[:], 1.0)
            nc.vector.memset(bn8_sb[:], -8.0)
            nc.vector.memset(eps_sb[:], EPS)
            nc.vector.memset(c26_sb[:], 2.62)
            nc.vector.memset(c15_sb[:], 1.5)

            # PE warmup during the initial load: ~2.7us of dummy matmuls so
            # HAM reaches K=8/8 before the first real QKV matmul.
            warm = pd_pool.tile([128, 128], F32, tag="pd", name="pd")
            for _ in range(26):
                nc.tensor.matmul(
                    warm[:], ones_sb[:], ones_sb[:], start=True, stop=True
                )

            # ---- stage B: QKV + rmsnorm(Newton) + rotary + batched transpose ----
            def emit_group(G):  # one group of 4 t-blocks
                qkraw = qkpool.tile([128, 4, 256], FP16, tag="qkraw")
                for j in range(4):
                    tb = 4 * G + j
                    pq = ppb_pool.tile([128, 384], F32, tag="ppb")
                    for kc in range(KC):
                        nc.tensor.matmul(
                            pq[:],
                            x_sb[:, G, kc, ts(j, 128)],
                            wqkv_sb[:, kc, :],
                            start=(kc == 0),
                            stop=(kc == KC - 1),
                        )
                    nc.scalar.copy(qkraw[:, j], pq[:, 0:256])
                    nc.scalar.copy(v_sb[:, tb, :], pq[:, 256:384])
                    sq = wpool.tile([128, 256], F32, tag="sq")
                    nc.scalar.square(sq[:], pq[:, 0:256])
                    nc.vector.reduce_sum(
                        ssq_sb[:, tb],
                        sq[:].rearrange("p (h c) -> p h c", c=HD),
                        axis=mybir.AxisListType.X,
                    )

                # Newton rsqrt for the group's 16 (tb, subhead) scales
                epsb = eps_sb[:].unsqueeze(2).broadcast_to([128, 4, 4])
                c26b = c26_sb[:].unsqueeze(2).broadcast_to([128, 4, 4])
                c15b = c15_sb[:].unsqueeze(2).broadcast_to([128, 4, 4])
                m = wpool.tile([128, 4, 4], F32, tag="m")
                nc.vector.scalar_tensor_tensor(
                    m[:], ssq_sb[:, ts(G, 4)], 1.0 / HD, epsb, ALU.mult, ALU.add
                )
                rsc = wpool.tile([128, 4, 4], F32, tag="rsc")
                nc.vector.scalar_tensor_tensor(
                    rsc[:], m[:], -2.22, c26b, ALU.mult, ALU.add
                )
                nc.vector.tensor_scalar(rsc[:], rsc[:], 0.8, 2.5, ALU.max, ALU.min)
                nt = wpool.tile([128, 4, 4], F32, tag="nt")
                for _ in range(3):
                    nc.vector.tensor_mul(nt[:], rsc[:], rsc[:])
                    nc.vector.tensor_mul(nt[:], nt[:], m[:])
                    nc.vector.scalar_tensor_tensor(
                        nt[:], nt[:], -0.5, c15b, ALU.mult, ALU.add
                    )
                    nc.vector.tensor_mul(rsc[:], rsc[:], nt[:])

                # batched rotary over the 4 t-blocks (q and k separately)
                cosb = (
                    cos_sb[:, ts(G, 4), :].unsqueeze(2).broadcast_to([128, 4, 2, 32])
                )
                sinb = (
                    sin_sb[:, ts(G, 4), :].unsqueeze(2).broadcast_to([128, 4, 2, 32])
                )
                stages = []
                for qk in range(2):  # 0: q, 1: k
                    eng = nc.vector
                    raw = qkraw[:, :, ds(qk * 128, 128)].rearrange(
                        "p a (h c) -> p a h c", c=HD
                    )
                    n1 = raw[:, :, :, 0:32]
                    n2 = raw[:, :, :, 32:64]
                    st = wpool.tile([128, 4, 2, HD], FP16, tag=f"st{qk}")
                    tmp = wpool.tile([128, 4, 2, 32], FP16, tag=f"rt1{qk}")
                    tmp2 = wpool.tile([128, 4, 2, 32], FP16, tag=f"rt2{qk}")
                    eng.tensor_mul(tmp[:], n1, cosb)
                    eng.tensor_mul(tmp2[:], n2, sinb)
                    eng.tensor_add(st[:, :, :, 0:32], tmp[:], tmp2[:])
                    eng.tensor_mul(tmp[:], n2, cosb)
                    eng.tensor_mul(tmp2[:], n1, sinb)
                    eng.tensor_sub(st[:, :, :, 32:64], tmp[:], tmp2[:])
                    rscb = (
                        rsc[:, :, ds(qk * 2, 2)]
                        .unsqueeze(3)
                        .broadcast_to([128, 4, 2, HD])
                    )
                    eng.tensor_mul(st[:], st[:], rscb)
                    stages.append(st)

                # batched DMA xbar transposes: [t, 4*128] -> 4 tiles of [c, t]
                nc.sync.dma_start_transpose(
                    qT_sb[:, ts(G, 512)].rearrange("p (a b) -> p a b", a=4),
                    stages[0][:].rearrange("p a h c -> p (a h c)"),
                )
                nc.sync.dma_start_transpose(
                    kT_sb[:, ts(G, 512)].rearrange("p (a b) -> p a b", a=4),
                    stages[1][:].rearrange("p a h c -> p (a h c)"),
                )

            # ---- stage C: differential attention + partial projection ----
            # The projection of chunk tc-1 is software-pipelined into the
            # attention si-loop of chunk tc (one i-half per s-block) so its
            # matmuls and PSUM evacuations fill the exp-wait gaps.
            def emit_proj_half(pc, tb4, half, ost, pool):
                po = pool.tile([128, 512], F32, tag="pd" if pool is pd_pool else "po")
                nc.tensor.matmul(
                    po[:],
                    ycomb_sb[:, ds(pc * 512 + tb4 * 128, 128)],
                    wpp_sb[:, ts(half, 512)],
                    start=True,
                    stop=True,
                )
                if half == 0:
                    nc.vector.tensor_copy(ost[:, tb4, 0:512], po[:])
                else:
                    nc.scalar.copy(ost[:, tb4, 512:1024], po[:])

            def emit_chunk(tc_i):
                nsb = 4 * tc_i + 4  # s-blocks touching this t-chunk
                pyp = pyp_pool.tile([128, 1024], F32, tag="pyp", name="pyp")
                pd0 = pd_pool.tile([128, 512], F32, tag="pd", name="pd")
                if tc_i:
                    ost_prev = opool.tile([128, 4, D], FP16, tag="ost", name="ost")
                else:
                    ost_prev = None
                for si in range(nsb):
                    col0 = max(0, si * 128 - tc_i * 512)
                    w = 512 - col0
                    ppb = ppb_pool.tile([128, 1024], F32, tag="ppb")
                    for g in range(2):
                        nc.tensor.matmul(
                            ppb[:, ds(512 * g + col0, w)],
                            kT_sb[ds(g * 64, 64), ts(si, 128)],
                            qT_sb[ds(g * 64, 64), ds(tc_i * 512 + col0, w)],
                            start=True,
                            stop=True,
                        )
                    pt = ppool.tile([128, 1024], FP16, tag="pt")
                    if col0 == 0:
                        nc.scalar.activation(
                            pt[:], ppb[:], AF.Exp, bias=bn8_sb[:], scale=SCALE
                        )
                    else:
                        for g in range(2):
                            nc.scalar.activation(
                                pt[:, ds(512 * g + col0, w)],
                                ppb[:, ds(512 * g + col0, w)],
                                AF.Exp,
                                bias=bn8_sb[:],
                                scale=SCALE,
                            )
                    if col0 > 0 or si * 128 == tc_i * 512:
                        # diagonal block: zero out s > t inside it (both groups)
                        dslc = (
                            pt[:]
                            .rearrange("p (g t) -> p g t", g=2)[:, :, ds(col0, 128)]
                        )
                        nc.vector.tensor_mul(
                            dslc,
                            dslc,
                            diag_sb[:].unsqueeze(1).broadcast_to([128, 2, 128]),
                        )
                    last = si == nsb - 1
                    for g in range(2):
                        nc.tensor.matmul(
                            pyp[:, ds(512 * g + col0, w)],
                            v_sb[:, si, :],
                            pt[:, ds(512 * g + col0, w)],
                            start=(si == 0),
                            stop=last,
                        )
                    # denom g0 on PE
                    nc.tensor.matmul(
                        pd0[:, ds(col0, w)],
                        ones_sb[:],
                        pt[:, ds(col0, w)],
                        start=(si == 0),
                        stop=last,
                    )
                    # denom g1 accumulates on gpsimd (off the critical path)
                    if si == 0:
                        nc.vector.tensor_copy(dacc1_sb[:], pt[:, 512:1024])
                    else:
                        nc.gpsimd.tensor_add(
                            dacc1_sb[:, ds(col0, w)],
                            dacc1_sb[:, ds(col0, w)],
                            pt[:, ds(512 + col0, w)],
                        )
                    # pipelined projection of the previous chunk
                    if tc_i and si < 8:
                        emit_proj_half(tc_i - 1, si // 2, si % 2, ost_prev, po_pool)
                        if si == 7:
                            nc.sync.dma_start(
                                out=outd[:, ts(tc_i - 1, 4 * D)],
                                in_=ost_prev[:].rearrange("p a b -> p (a b)"),
                            )

                rec0 = wpool.tile([128, 512], F32, tag="rec0")
                nc.vector.reciprocal_approx_fast(rec0[:], pd0[:])
                pdb1 = po_pool.tile([128, 512], F32, tag="po", name="po")
                nc.tensor.matmul(
                    pdb1[:], ones_sb[:], dacc1_sb[:], start=True, stop=True
                )
                rec1 = wpool.tile([128, 512], F32, tag="rec1")
                nc.vector.reciprocal_approx_fast(rec1[:], pdb1[:])
                yn0 = wpool.tile([128, 512], F32, tag="yn0")
                nc.vector.tensor_mul(yn0[:], pyp[:, 0:512], rec0[:])
                yn1 = wpool.tile([128, 512], F32, tag="yn1")
                nc.vector.tensor_mul(yn1[:], pyp[:, 512:1024], rec1[:])
                nc.vector.scalar_tensor_tensor(
                    ycomb_sb[:, ts(tc_i, 512)],
                    yn1[:],
                    -lam,
                    yn0[:],
                    ALU.mult,
                    ALU.add,
                )

            for G in range(4):
                emit_group(G)
            for tc_i in range(NTC):
                emit_chunk(tc_i)

            # final chunk's projection (alternating the two free PSUM banks)
            ost = opool.tile([128, 4, D], FP16, tag="ost")
            for tb4 in range(4):
                for half in range(2):
                    pool = po_pool if (tb4 * 2 + half) % 2 == 0 else pd_pool
                    emit_proj_half(NTC - 1, tb4, half, ost, pool)
                nc.sync.dma_start(
                    out=outd[
                        :, (NTC - 1) * 4 * D + tb4 * D : (NTC - 1) * 4 * D + (tb4 + 1) * D
                    ],
                    in_=ost[:, tb4].rearrange("p b -> p b"),
                )

    nc.compile()
    return nc


def _make_in_maps(x, Wq, Wk, Wv, Wproj):
    f16 = np.float16
    xT = np.ascontiguousarray(x[0].T).astype(f16)  # [D, T]
    # x repack: [p, tc, kc, u] = x[0, tc*512+u, kc*128+p]
    xr = xT.reshape(KC, 128, NTC, 512).transpose(1, 2, 0, 3).reshape(128, -1)
    xr = np.ascontiguousarray(xr)

    # rotary tables, [tp, tb, 32] layout, cos|sin concatenated
    inv = 1.0 / (10000.0 ** (np.arange(0, HD, 2, dtype=np.float32) / HD))
    fr = np.outer(np.arange(T, dtype=np.float32), inv)  # [T, 32]
    cos = np.cos(fr).reshape(TB, 128, 32).transpose(1, 0, 2).reshape(128, -1)
    sin = np.sin(fr).reshape(TB, 128, 32).transpose(1, 0, 2).reshape(128, -1)
    cost = np.concatenate([cos, sin], axis=1).astype(f16)
    diag = np.triu(np.ones((128, 128), np.float32)).astype(f16)

    in_maps = []
    for h in range(NCORES):
        wcat = np.concatenate(
            [
                Wq[h * 64 : h * 64 + 64],
                Wq[512 + h * 64 : 512 + h * 64 + 64],
                Wk[h * 64 : h * 64 + 64],
                Wk[512 + h * 64 : 512 + h * 64 + 64],
                Wv[h * 128 : h * 128 + 128],
            ],
            axis=0,
        ).T  # [D, 384]
        wqkv = wcat.reshape(KC, 128, 384).transpose(1, 0, 2).reshape(128, -1)
        wpp = Wproj[:, h * 128 : (h + 1) * 128].T  # [j, i]
        in_maps.append(
            {
                "x": xr,
                "wqkv": np.ascontiguousarray(wqkv).astype(f16),
                "wpp": np.ascontiguousarray(wpp).astype(f16),
                "cost": cost,
                "diag": diag,
            }
        )
    return in_maps


def _get_program(lam: float):
    key = round(lam, 10)
    if key not in _CACHE:
        _CACHE[key] = _build_program(lam)
    return _CACHE[key]


def kernel(x, Wq, Wk, Wv, Wproj, lambda_q1, lambda_k1, lambda_q2, lambda_k2):
    x = np.asarray(x, np.float32)
    Wq, Wk = np.asarray(Wq, np.float32), np.asarray(Wk, np.float32)
    Wv, Wproj = np.asarray(Wv, np.float32), np.asarray(Wproj, np.float32)

    lam1 = float(np.exp(np.sum(np.asarray(lambda_q1) * np.asarray(lambda_k1))))
    lam2 = float(np.exp(np.sum(np.asarray(lambda_q2) * np.asarray(lambda_k2))))
    lam = lam1 - lam2 + LAMBDA_INIT

    in_maps = _make_in_maps(x, Wq, Wk, Wv, Wproj)
    nc = _get_program(lam)

    res = run_bass_kernel_spmd(nc, in_maps, list(range(NCORES)))
    acc = res.results[0]["out"].astype(np.float32)
    for h in range(1, NCORES):
        acc += res.results[h]["out"].astype(np.float32)
    y = acc.reshape(128, NTC, 4, D).transpose(1, 2, 0, 3).reshape(1, T, D)
    return np.ascontiguousarray(y)


if __name__ == "__main__":
    rng = np.random.default_rng(0)
    ins = {
        "x": rng.standard_normal((1, T, D), np.float32),
        "Wq": (rng.standard_normal((D, D)) * 0.02).astype(np.float32),
        "Wk": (rng.standard_normal((D, D)) * 0.02).astype(np.float32),
        "Wv": (rng.standard_normal((D, D)) * 0.02).astype(np.float32),
        "Wproj": (rng.standard_normal((D, D)) * 0.02).astype(np.float32),
        "lambda_q1": (rng.standard_normal(32) * 0.1).astype(np.float32),
        "lambda_k1": (rng.standard_normal(32) * 0.1).astype(np.float32),
        "lambda_q2": (rng.standard_normal(32) * 0.1).astype(np.float32),
        "lambda_k2": (rng.standard_normal(32) * 0.1).astype(np.float32),
    }
    y = kernel(**ins)
    print("kernel output", y.shape, y.dtype, float(np.abs(y).mean()))


# revision 41
# speedup vs baseline: 1.0614x; 1.0129x over previous
"""Trainium2 Bass kernel for MixerDiffAttention (differential attention).

Sharding: tensor-parallel over the 8 (n_head//2) head groups across 8 cores.
Each core computes QKV projections for its head group, both differential
attention branches, the normalized combination y1 - lambda*y2, and its head's
partial product with the row-sharded c_proj. The host sums the 8 partial
outputs.

v3 notes:
  - All tensors fp16 on-chip; every DMA is per-partition contiguous.
  - Stage B works in groups of 4 t-blocks: QKV matmuls -> ACT stages raw q/k
    to SBUF -> DVE computes the rms scale with a Newton rsqrt (linear init,
    clipped, 3 iterations) so the only ACT table set ever loaded is exp's ->
    batched rotary -> one batched DMA xbar transpose per group for qT/kT.
  - scores exp'ed with bias -8 (|score*scale| <= 8 after rms norm) into fp16.
  - softmax denominators: group 0 accumulates on PE (ones-matmul per s-block),
    group 1 on DVE in fp16 (2x packed); one ones-matmul per t-chunk reduces
    group 1 across partitions.
  - partial projection uses ycomb chunks as the stationary operand, giving
    [t, i]-layout outputs; PSUM evacuation alternates DVE/ACT; projection
    PSUM is double-buffered via the pd pool ring.
"""

import os
import sys

import numpy as np

for _p in ("/opt/trn_rl_repo", "/root/.axon_site/_ro/trn_rl_repo"):
    if os.path.isdir(_p) and _p not in sys.path:
        sys.path.insert(0, _p)

import concourse.bass as bass
import concourse.mybir as mybir
import concourse.tile as tile
from concourse import bacc
from concourse.bass import ds, ts
from concourse.bass_utils import run_bass_kernel_spmd

FP16 = mybir.dt.float16
F32 = mybir.dt.float32
AF = mybir.ActivationFunctionType
ALU = mybir.AluOpType

N_HEAD = 16
D = 1024
HD = 64  # head dim
T = 2048
NCORES = 8
TB = T // 128  # 16 t-blocks
KC = D // 128  # 8 contraction chunks
NTC = T // 512  # 4 t-chunks of 512
LAMBDA_INIT = 0.8 - 0.6 * float(np.exp(-0.3 * 1))
EPS = float(np.finfo(np.float32).eps)
SCALE = 1.0 / 8.0  # 1/sqrt(64)

_CACHE = {}


def _build_program(lam: float) -> bass.Bass:
    nc = bacc.Bacc("TRN2", target_bir_lowering=False, debug=False)

    xd = nc.declare_dram_parameter("x", [128, NTC * KC * 512], FP16, isOutput=False)
    wqkvd = nc.declare_dram_parameter("wqkv", [128, KC * 384], FP16, isOutput=False)
    wppd = nc.declare_dram_parameter("wpp", [128, D], FP16, isOutput=False)
    costd = nc.declare_dram_parameter("cost", [128, 2 * TB * 32], FP16, isOutput=False)
    diagd = nc.declare_dram_parameter("diag", [128, 128], FP16, isOutput=False)
    outd = nc.declare_dram_parameter("out", [128, NTC * 4 * D], FP16, isOutput=True)

    with tile.TileContext(nc) as tc:
        with (
            tc.tile_pool(name="const", bufs=1) as cpool,
            tc.tile_pool(name="work", bufs=4) as wpool,
            tc.tile_pool(name="qk", bufs=2) as qkpool,
            tc.tile_pool(name="ptile", bufs=6) as ppool,
            tc.tile_pool(name="ostage", bufs=2) as opool,
            tc.tile_pool(name="ppb", bufs=2, space="PSUM") as ppb_pool,
            tc.tile_pool(name="pyp", bufs=1, space="PSUM") as pyp_pool,
            tc.tile_pool(name="pd", bufs=1, space="PSUM") as pd_pool,
            tc.tile_pool(name="po", bufs=1, space="PSUM") as po_pool,
        ):
            # ---- persistent SBUF ----
            x_sb = cpool.tile([128, NTC, KC, 512], FP16, tag="x")
            wqkv_sb = cpool.tile([128, KC, 384], FP16, tag="wqkv")
            wpp_sb = cpool.tile([128, D], FP16, tag="wpp")
            cos_sb = cpool.tile([128, TB, 32], FP16, tag="cos")
            sin_sb = cpool.tile([128, TB, 32], FP16, tag="sin")
            diag_sb = cpool.tile([128, 128], FP16, tag="diag")
            ones_sb = cpool.tile([128, 128], FP16, tag="ones")
            qT_sb = cpool.tile([128, T], FP16, tag="qT")  # rows 0:64 g0, 64:128 g1
            kT_sb = cpool.tile([128, T], FP16, tag="kT")
            v_sb = cpool.tile([128, TB, 128], FP16, tag="v")  # [s-part, tb, j]
            ycomb_sb = cpool.tile([128, T], FP16, tag="ycomb")  # [j, t]
            dacc1_sb = cpool.tile([128, 512], FP16, tag="dacc1")
            ssq_sb = cpool.tile([128, TB, 4], F32, tag="ssq")
            bn8_sb = cpool.tile([128, 1], F32, tag="bn8")
            eps_sb = cpool.tile([128, 1], F32, tag="eps")
            c26_sb = cpool.tile([128, 1], F32, tag="c26")
            c15_sb = cpool.tile([128, 1], F32, tag="c15")

            # ---- loads (few, contiguous; first QKV gated on lo-halves only) ----
            nc.sync.dma_start(
                out=wqkv_sb[:, 0:4].rearrange("p a b -> p (a b)"),
                in_=wqkvd[:, 0 : 4 * 384],
            )
            nc.scalar.dma_start(
                out=x_sb[:, 0, 0:4].rearrange("p a b -> p (a b)"),
                in_=xd[:, 0:2048],
            )
            nc.sync.dma_start(
                out=wqkv_sb[:, 4:8].rearrange("p a b -> p (a b)"),
                in_=wqkvd[:, 4 * 384 : 8 * 384],
            )
            nc.scalar.dma_start(
                out=x_sb[:, 0, 4:8].rearrange("p a b -> p (a b)"),
                in_=xd[:, 2048:4096],
            )
            for tc_i in range(1, NTC):
                nc.sync.dma_start(
                    out=x_sb[:, tc_i].rearrange("p a b -> p (a b)"),
                    in_=xd[:, ts(tc_i, KC * 512)],
                )
            nc.gpsimd.dma_start(
                out=cos_sb[:].rearrange("p a b -> p (a b)"), in_=costd[:, 0 : TB * 32]
            )
            nc.gpsimd.dma_start(
                out=sin_sb[:].rearrange("p a b -> p (a b)"),
                in_=costd[:, TB * 32 : 2 * TB * 32],
            )
            nc.gpsimd.dma_start(out=diag_sb[:], in_=diagd[:, :])
            nc.gpsimd.dma_start(out=wpp_sb[:], in_=wppd[:, :])
            nc.vector.memset(ones_sb[:], 1.0)
            nc.vector.memset(bn8_sb[:], -8.0)
            nc.vector.memset(eps_sb[:], EPS)
            nc.vector.memset(c26_sb[:], 2.62)
            nc.vector.memset(c15_sb[:], 1.5)

            # PE warmup during the initial load: ~2.7us of dummy matmuls so
            # HAM reaches K=8/8 before the first real QKV matmul.
            warm = pd_pool.tile([128, 128], F32, tag="pd", name="pd")
            for _ in range(26):
                nc.tensor.matmul(
                    warm[:], ones_sb[:], ones_sb[:], start=True, stop=True
                )

            # ---- stage B: QKV + rmsnorm(Newton) + rotary + batched transpose ----
            def emit_group(G):  # one group of 4 t-blocks
                qkraw = qkpool.tile([128, 4, 256], FP16, tag="qkraw")
                for j in range(4):
                    tb = 4 * G + j
                    pq = ppb_pool.tile([128, 384], F32, tag="ppb")
                    for kc in range(KC):
                        nc.tensor.matmul(
                            pq[:],
                            x_sb[:, G, kc, ts(j, 128)],
                            wqkv_sb[:, kc, :],
                            start=(kc == 0),
                            stop=(kc == KC - 1),
                        )
                    nc.scalar.copy(qkraw[:, j], pq[:, 0:256])
                    nc.scalar.copy(v_sb[:, tb, :], pq[:, 256:384])
                    sq = wpool.tile([128, 256], F32, tag="sq")
                    nc.gpsimd.tensor_mul(sq[:], qkraw[:, j], qkraw[:, j])
                    nc.vector.reduce_sum(
                        ssq_sb[:, tb],
                        sq[:].rearrange("p (h c) -> p h c", c=HD),
                        axis=mybir.AxisListType.X,
                    )

                # Newton rsqrt for the group's 16 (tb, subhead) scales
                epsb = eps_sb[:].unsqueeze(2).broadcast_to([128, 4, 4])
                c26b = c26_sb[:].unsqueeze(2).broadcast_to([128, 4, 4])
                c15b = c15_sb[:].unsqueeze(2).broadcast_to([128, 4, 4])
                m = wpool.tile([128, 4, 4], F32, tag="m")
                nc.vector.scalar_tensor_tensor(
                    m[:], ssq_sb[:, ts(G, 4)], 1.0 / HD, epsb, ALU.mult, ALU.add
                )
                rsc = wpool.tile([128, 4, 4], F32, tag="rsc")
                nc.vector.scalar_tensor_tensor(
                    rsc[:], m[:], -2.22, c26b, ALU.mult, ALU.add
                )
                nc.vector.tensor_scalar(rsc[:], rsc[:], 0.8, 2.5, ALU.max, ALU.min)
                nt = wpool.tile([128, 4, 4], F32, tag="nt")
                for _ in range(3):
                    nc.vector.tensor_mul(nt[:], rsc[:], rsc[:])
                    nc.vector.tensor_mul(nt[:], nt[:], m[:])
                    nc.vector.scalar_tensor_tensor(
                        nt[:], nt[:], -0.5, c15b, ALU.mult, ALU.add
                    )
                    nc.vector.tensor_mul(rsc[:], rsc[:], nt[:])

                # batched rotary over the 4 t-blocks (q and k separately)
                cosb = (
                    cos_sb[:, ts(G, 4), :].unsqueeze(2).broadcast_to([128, 4, 2, 32])
                )
                sinb = (
                    sin_sb[:, ts(G, 4), :].unsqueeze(2).broadcast_to([128, 4, 2, 32])
                )
                stages = []
                for qk in range(2):  # 0: q, 1: k
                    eng = nc.vector
                    raw = qkraw[:, :, ds(qk * 128, 128)].rearrange(
                        "p a (h c) -> p a h c", c=HD
                    )
                    n1 = raw[:, :, :, 0:32]
                    n2 = raw[:, :, :, 32:64]
                    st = wpool.tile([128, 4, 2, HD], FP16, tag=f"st{qk}")
                    tmp = wpool.tile([128, 4, 2, 32], FP16, tag=f"rt1{qk}")
                    tmp2 = wpool.tile([128, 4, 2, 32], FP16, tag=f"rt2{qk}")
                    eng.tensor_mul(tmp[:], n1, cosb)
                    eng.tensor_mul(tmp2[:], n2, sinb)
                    eng.tensor_add(st[:, :, :, 0:32], tmp[:], tmp2[:])
                    eng.tensor_mul(tmp[:], n2, cosb)
                    eng.tensor_mul(tmp2[:], n1, sinb)
                    eng.tensor_sub(st[:, :, :, 32:64], tmp[:], tmp2[:])
                    rscb = (
                        rsc[:, :, ds(qk * 2, 2)]
                        .unsqueeze(3)
                        .broadcast_to([128, 4, 2, HD])
                    )
                    eng.tensor_mul(st[:], st[:], rscb)
                    stages.append(st)

                # batched DMA xbar transposes: [t, 4*128] -> 4 tiles of [c, t]
                nc.sync.dma_start_transpose(
                    qT_sb[:, ts(G, 512)].rearrange("p (a b) -> p a b", a=4),
                    stages[0][:].rearrange("p a h c -> p (a h c)"),
                )
                nc.sync.dma_start_transpose(
                    kT_sb[:, ts(G, 512)].rearrange("p (a b) -> p a b", a=4),
                    stages[1][:].rearrange("p a h c -> p (a h c)"),
                )

            # ---- stage C: differential attention + partial projection ----
            # The projection of chunk tc-1 is software-pipelined into the
            # attention si-loop of chunk tc (one i-half per s-block) so its
            # matmuls and PSUM evacuations fill the exp-wait gaps.
            def emit_proj_half(pc, tb4, half, ost, pool):
                po = pool.tile([128, 512], F32, tag="pd" if pool is pd_pool else "po")
                nc.tensor.matmul(
                    po[:],
                    ycomb_sb[:, ds(pc * 512 + tb4 * 128, 128)],
                    wpp_sb[:, ts(half, 512)],
                    start=True,
                    stop=True,
                )
                nc.vector.tensor_copy(ost[:, tb4, ds(half * 512, 512)], po[:])

            def emit_chunk(tc_i):
                nsb = 4 * tc_i + 4  # s-blocks touching this t-chunk
                pyp = pyp_pool.tile([128, 1024], F32, tag="pyp", name="pyp")
                pd0 = pd_pool.tile([128, 512], F32, tag="pd", name="pd")
                if tc_i:
                    ost_prev = opool.tile([128, 4, D], FP16, tag="ost", name="ost")
                else:
                    ost_prev = None
                for si in range(nsb):
                    col0 = max(0, si * 128 - tc_i * 512)
                    w = 512 - col0
                    ppb = ppb_pool.tile([128, 1024], F32, tag="ppb")
                    for g in range(2):
                        nc.tensor.matmul(
                            ppb[:, ds(512 * g + col0, w)],
                            kT_sb[ds(g * 64, 64), ts(si, 128)],
                            qT_sb[ds(g * 64, 64), ds(tc_i * 512 + col0, w)],
                            start=True,
                            stop=True,
                        )
                    pt = ppool.tile([128, 1024], FP16, tag="pt")
                    if col0 == 0:
                        nc.scalar.activation(
                            pt[:], ppb[:], AF.Exp, bias=bn8_sb[:], scale=SCALE
                        )
                    else:
                        for g in range(2):
                            nc.scalar.activation(
                                pt[:, ds(512 * g + col0, w)],
                                ppb[:, ds(512 * g + col0, w)],
                                AF.Exp,
                                bias=bn8_sb[:],
                                scale=SCALE,
                            )
                    if col0 > 0 or si * 128 == tc_i * 512:
                        # diagonal block: zero out s > t inside it (both groups)
                        dslc = (
                            pt[:]
                            .rearrange("p (g t) -> p g t", g=2)[:, :, ds(col0, 128)]
                        )
                        nc.vector.tensor_mul(
                            dslc,
                            dslc,
                            diag_sb[:].unsqueeze(1).broadcast_to([128, 2, 128]),
                        )
                    last = si == nsb - 1
                    for g in range(2):
                        nc.tensor.matmul(
                            pyp[:, ds(512 * g + col0, w)],
                            v_sb[:, si, :],
                            pt[:, ds(512 * g + col0, w)],
                            start=(si == 0),
                            stop=last,
                        )
                    # denom g0 on PE
                    nc.tensor.matmul(
                        pd0[:, ds(col0, w)],
                        ones_sb[:],
                        pt[:, ds(col0, w)],
                        start=(si == 0),
                        stop=last,
                    )
                    # denom g1 accumulates on gpsimd (off the critical path)
                    if si == 0:
                        nc.vector.tensor_copy(dacc1_sb[:], pt[:, 512:1024])
                    else:
                        nc.gpsimd.tensor_add(
                            dacc1_sb[:, ds(col0, w)],
                            dacc1_sb[:, ds(col0, w)],
                            pt[:, ds(512 + col0, w)],
                        )
                    # pipelined projection of the previous chunk
                    if tc_i and si < 8:
                        emit_proj_half(tc_i - 1, si // 2, si % 2, ost_prev, po_pool)
                        if si == 7:
                            nc.sync.dma_start(
                                out=outd[:, ts(tc_i - 1, 4 * D)],
                                in_=ost_prev[:].rearrange("p a b -> p (a b)"),
                            )

                rec0 = wpool.tile([128, 512], F32, tag="rec0")
                nc.vector.reciprocal_approx_fast(rec0[:], pd0[:])
                pdb1 = po_pool.tile([128, 512], F32, tag="po", name="po")
                nc.tensor.matmul(
                    pdb1[:], ones_sb[:], dacc1_sb[:], start=True, stop=True
                )
                rec1 = wpool.tile([128, 512], F32, tag="rec1")
                nc.vector.reciprocal_approx_fast(rec1[:], pdb1[:])
                yn0 = wpool.tile([128, 512], F32, tag="yn0")
                nc.vector.tensor_mul(yn0[:], pyp[:, 0:512], rec0[:])
                yn1 = wpool.tile([128, 512], F32, tag="yn1")
                nc.vector.tensor_mul(yn1[:], pyp[:, 512:1024], rec1[:])
                nc.vector.scalar_tensor_tensor(
                    ycomb_sb[:, ts(tc_i, 512)],
                    yn1[:],
                    -lam,
                    yn0[:],
                    ALU.mult,
                    ALU.add,
                )

            for G in range(4):
                emit_group(G)
            for tc_i in range(NTC):
                emit_chunk(tc_i)

            # final chunk's projection (alternating the two free PSUM banks)
            ost = opool.tile([128, 4, D], FP16, tag="ost")
            for tb4 in range(4):
                for half in range(2):
                    pool = po_pool if (tb4 * 2 + half) % 2 == 0 else pd_pool
                    emit_proj_half(NTC - 1, tb4, half, ost, pool)
                nc.sync.dma_start(
                    out=outd[
                        :, (NTC - 1) * 4 * D + tb4 * D : (NTC - 1) * 4 * D + (tb4 + 1) * D
                    ],
                    in_=ost[:, tb4].rearrange("p b -> p b"),
                )

    nc.compile()
    return nc


def _make_in_maps(x, Wq, Wk, Wv, Wproj):
    f16 = np.float16
    xT = np.ascontiguousarray(x[0].T).astype(f16)  # [D, T]
    # x repack: [p, tc, kc, u] = x[0, tc*512+u, kc*128+p]
    xr = xT.reshape(KC, 128, NTC, 512).transpose(1, 2, 0, 3).reshape(128, -1)
    xr = np.ascontiguousarray(xr)

    # rotary tables, [tp, tb, 32] layout, cos|sin concatenated
    inv = 1.0 / (10000.0 ** (np.arange(0, HD, 2, dtype=np.float32) / HD))
    fr = np.outer(np.arange(T, dtype=np.float32), inv)  # [T, 32]
    cos = np.cos(fr).reshape(TB, 128, 32).transpose(1, 0, 2).reshape(128, -1)
    sin = np.sin(fr).reshape(TB, 128, 32).transpose(1, 0, 2).reshape(128, -1)
    cost = np.concatenate([cos, sin], axis=1).astype(f16)
    diag = np.triu(np.ones((128, 128), np.float32)).astype(f16)

    in_maps = []
    for h in range(NCORES):
        wcat = np.concatenate(
            [
                Wq[h * 64 : h * 64 + 64],
                Wq[512 + h * 64 : 512 + h * 64 + 64],
                Wk[h * 64 : h * 64 + 64],
                Wk[512 + h * 64 : 512 + h * 64 + 64],
                Wv[h * 128 : h * 128 + 128],
            ],
            axis=0,
        ).T  # [D, 384]
        wqkv = wcat.reshape(KC, 128, 384).transpose(1, 0, 2).reshape(128, -1)
        wpp = Wproj[:, h * 128 : (h + 1) * 128].T  # [j, i]
        in_maps.append(
            {
                "x": xr,
                "wqkv": np.ascontiguousarray(wqkv).astype(f16),
                "wpp": np.ascontiguousarray(wpp).astype(f16),
                "cost": cost,
                "diag": diag,
            }
        )
    return in_maps


def _get_program(lam: float):
    key = round(lam, 10)
    if key not in _CACHE:
        _CACHE[key] = _build_program(lam)
    return _CACHE[key]


def kernel(x, Wq, Wk, Wv, Wproj, lambda_q1, lambda_k1, lambda_q2, lambda_k2):
    x = np.asarray(x, np.float32)
    Wq, Wk = np.asarray(Wq, np.float32), np.asarray(Wk, np.float32)
    Wv, Wproj = np.asarray(Wv, np.float32), np.asarray(Wproj, np.float32)

    lam1 = float(np.exp(np.sum(np.asarray(lambda_q1) * np.asarray(lambda_k1))))
    lam2 = float(np.exp(np.sum(np.asarray(lambda_q2) * np.asarray(lambda_k2))))
    lam = lam1 - lam2 + LAMBDA_INIT

    in_maps = _make_in_maps(x, Wq, Wk, Wv, Wproj)
    nc = _get_program(lam)

    res = run_bass_kernel_spmd(nc, in_maps, list(range(NCORES)))
    acc = res.results[0]["out"].astype(np.float32)
    for h in range(1, NCORES):
        acc += res.results[h]["out"].astype(np.float32)
    y = acc.reshape(128, NTC, 4, D).transpose(1, 2, 0, 3).reshape(1, T, D)
    return np.ascontiguousarray(y)


if __name__ == "__main__":
    rng = np.random.default_rng(0)
    ins = {
        "x": rng.standard_normal((1, T, D), np.float32),
        "Wq": (rng.standard_normal((D, D)) * 0.02).astype(np.float32),
        "Wk": (rng.standard_normal((D, D)) * 0.02).astype(np.float32),
        "Wv": (rng.standard_normal((D, D)) * 0.02).astype(np.float32),
        "Wproj": (rng.standard_normal((D, D)) * 0.02).astype(np.float32),
        "lambda_q1": (rng.standard_normal(32) * 0.1).astype(np.float32),
        "lambda_k1": (rng.standard_normal(32) * 0.1).astype(np.float32),
        "lambda_q2": (rng.standard_normal(32) * 0.1).astype(np.float32),
        "lambda_k2": (rng.standard_normal(32) * 0.1).astype(np.float32),
    }
    y = kernel(**ins)
    print("kernel output", y.shape, y.dtype, float(np.abs(y).mean()))


# revision 42
# speedup vs baseline: 1.1045x; 1.0407x over previous
"""Trainium2 Bass kernel for MixerDiffAttention (differential attention).

Sharding: tensor-parallel over the 8 (n_head//2) head groups across 8 cores.
Each core computes QKV projections for its head group, both differential
attention branches, the normalized combination y1 - lambda*y2, and its head's
partial product with the row-sharded c_proj. The host sums the 8 partial
outputs.

v3 notes:
  - All tensors fp16 on-chip; every DMA is per-partition contiguous.
  - Stage B works in groups of 4 t-blocks: QKV matmuls -> ACT stages raw q/k
    to SBUF -> DVE computes the rms scale with a Newton rsqrt (linear init,
    clipped, 3 iterations) so the only ACT table set ever loaded is exp's ->
    batched rotary -> one batched DMA xbar transpose per group for qT/kT.
  - scores exp'ed with bias -8 (|score*scale| <= 8 after rms norm) into fp16.
  - softmax denominators: group 0 accumulates on PE (ones-matmul per s-block),
    group 1 on DVE in fp16 (2x packed); one ones-matmul per t-chunk reduces
    group 1 across partitions.
  - partial projection uses ycomb chunks as the stationary operand, giving
    [t, i]-layout outputs; PSUM evacuation alternates DVE/ACT; projection
    PSUM is double-buffered via the pd pool ring.
"""

import os
import sys

import numpy as np

for _p in ("/opt/trn_rl_repo", "/root/.axon_site/_ro/trn_rl_repo"):
    if os.path.isdir(_p) and _p not in sys.path:
        sys.path.insert(0, _p)

import concourse.bass as bass
import concourse.mybir as mybir
import concourse.tile as tile
from concourse import bacc
from concourse.bass import ds, ts
from concourse.bass_utils import run_bass_kernel_spmd

FP16 = mybir.dt.float16
F32 = mybir.dt.float32
AF = mybir.ActivationFunctionType
ALU = mybir.AluOpType

N_HEAD = 16
D = 1024
HD = 64  # head dim
T = 2048
NCORES = 8
TB = T // 128  # 16 t-blocks
KC = D // 128  # 8 contraction chunks
NTC = T // 512  # 4 t-chunks of 512
LAMBDA_INIT = 0.8 - 0.6 * float(np.exp(-0.3 * 1))
EPS = float(np.finfo(np.float32).eps)
SCALE = 1.0 / 8.0  # 1/sqrt(64)

_CACHE = {}


def _build_program(lam: float) -> bass.Bass:
    nc = bacc.Bacc("TRN2", target_bir_lowering=False, debug=False)

    xd = nc.declare_dram_parameter("x", [128, NTC * KC * 512], FP16, isOutput=False)
    wqkvd = nc.declare_dram_parameter("wqkv", [128, KC * 384], FP16, isOutput=False)
    wppd = nc.declare_dram_parameter("wpp", [128, D], FP16, isOutput=False)
    costd = nc.declare_dram_parameter("cost", [128, 2 * TB * 32], FP16, isOutput=False)
    diagd = nc.declare_dram_parameter("diag", [128, 128], FP16, isOutput=False)
    outd = nc.declare_dram_parameter("out", [128, NTC * 4 * D], FP16, isOutput=True)

    with tile.TileContext(nc) as tc:
        with (
            tc.tile_pool(name="const", bufs=1) as cpool,
            tc.tile_pool(name="work", bufs=4) as wpool,
            tc.tile_pool(name="qk", bufs=2) as qkpool,
            tc.tile_pool(name="ptile", bufs=6) as ppool,
            tc.tile_pool(name="ostage", bufs=2) as opool,
            tc.tile_pool(name="ppb", bufs=2, space="PSUM") as ppb_pool,
            tc.tile_pool(name="pyp", bufs=1, space="PSUM") as pyp_pool,
            tc.tile_pool(name="pd", bufs=1, space="PSUM") as pd_pool,
            tc.tile_pool(name="po", bufs=1, space="PSUM") as po_pool,
        ):
            # ---- persistent SBUF ----
            x_sb = cpool.tile([128, NTC, KC, 512], FP16, tag="x")
            wqkv_sb = cpool.tile([128, KC, 384], FP16, tag="wqkv")
            wpp_sb = cpool.tile([128, D], FP16, tag="wpp")
            cos_sb = cpool.tile([128, TB, 32], FP16, tag="cos")
            sin_sb = cpool.tile([128, TB, 32], FP16, tag="sin")
            diag_sb = cpool.tile([128, 128], FP16, tag="diag")
            ones_sb = cpool.tile([128, 128], FP16, tag="ones")
            qT_sb = cpool.tile([128, T], FP16, tag="qT")  # rows 0:64 g0, 64:128 g1
            kT_sb = cpool.tile([128, T], FP16, tag="kT")
            v_sb = cpool.tile([128, TB, 128], FP16, tag="v")  # [s-part, tb, j]
            ycomb_sb = cpool.tile([128, T], FP16, tag="ycomb")  # [j, t]
            dacc1_sb = cpool.tile([128, 512], FP16, tag="dacc1")
            ssq_sb = cpool.tile([128, TB, 4], F32, tag="ssq")
            bn8_sb = cpool.tile([128, 1], F32, tag="bn8")
            eps_sb = cpool.tile([128, 1], F32, tag="eps")
            c26_sb = cpool.tile([128, 1], F32, tag="c26")
            c15_sb = cpool.tile([128, 1], F32, tag="c15")

            # ---- loads (few, contiguous; first QKV gated on lo-halves only) ----
            nc.sync.dma_start(
                out=wqkv_sb[:, 0:4].rearrange("p a b -> p (a b)"),
                in_=wqkvd[:, 0 : 4 * 384],
            )
            nc.scalar.dma_start(
                out=x_sb[:, 0, 0:4].rearrange("p a b -> p (a b)"),
                in_=xd[:, 0:2048],
            )
            nc.sync.dma_start(
                out=wqkv_sb[:, 4:8].rearrange("p a b -> p (a b)"),
                in_=wqkvd[:, 4 * 384 : 8 * 384],
            )
            nc.scalar.dma_start(
                out=x_sb[:, 0, 4:8].rearrange("p a b -> p (a b)"),
                in_=xd[:, 2048:4096],
            )
            for tc_i in range(1, NTC):
                nc.sync.dma_start(
                    out=x_sb[:, tc_i].rearrange("p a b -> p (a b)"),
                    in_=xd[:, ts(tc_i, KC * 512)],
                )
            nc.gpsimd.dma_start(
                out=cos_sb[:].rearrange("p a b -> p (a b)"), in_=costd[:, 0 : TB * 32]
            )
            nc.gpsimd.dma_start(
                out=sin_sb[:].rearrange("p a b -> p (a b)"),
                in_=costd[:, TB * 32 : 2 * TB * 32],
            )
            nc.gpsimd.dma_start(out=diag_sb[:], in_=diagd[:, :])
            nc.gpsimd.dma_start(out=wpp_sb[:], in_=wppd[:, :])
            nc.vector.memset(ones_sb[:], 1.0)
            nc.vector.memset(bn8_sb[:], -8.0)
            nc.vector.memset(eps_sb[:], EPS)
            nc.vector.memset(c26_sb[:], 2.62)
            nc.vector.memset(c15_sb[:], 1.5)

            # PE warmup during the initial load: ~2.7us of dummy matmuls so
            # HAM reaches K=8/8 before the first real QKV matmul.
            warm = pd_pool.tile([128, 128], F32, tag="pd", name="pd")
            for _ in range(26):
                nc.tensor.matmul(
                    warm[:], ones_sb[:], ones_sb[:], start=True, stop=True
                )

            # ---- stage B: QKV + rmsnorm(Newton) + rotary + batched transpose ----
            def emit_group(G):  # one group of 4 t-blocks
                qkraw = qkpool.tile([128, 4, 256], FP16, tag="qkraw")
                for j in range(4):
                    tb = 4 * G + j
                    pq = ppb_pool.tile([128, 384], F32, tag="ppb")
                    for kc in range(KC):
                        nc.tensor.matmul(
                            pq[:],
                            x_sb[:, G, kc, ts(j, 128)],
                            wqkv_sb[:, kc, :],
                            start=(kc == 0),
                            stop=(kc == KC - 1),
                        )
                    nc.scalar.copy(qkraw[:, j], pq[:, 0:256])
                    nc.scalar.copy(v_sb[:, tb, :], pq[:, 256:384])
                    sq = wpool.tile([128, 256], F32, tag="sq")
                    nc.scalar.square(sq[:], pq[:, 0:256])
                    nc.vector.reduce_sum(
                        ssq_sb[:, tb],
                        sq[:].rearrange("p (h c) -> p h c", c=HD),
                        axis=mybir.AxisListType.X,
                    )

                # Newton rsqrt for the group's 16 (tb, subhead) scales
                epsb = eps_sb[:].unsqueeze(2).broadcast_to([128, 4, 4])
                c26b = c26_sb[:].unsqueeze(2).broadcast_to([128, 4, 4])
                c15b = c15_sb[:].unsqueeze(2).broadcast_to([128, 4, 4])
                m = wpool.tile([128, 4, 4], F32, tag="m")
                nc.vector.scalar_tensor_tensor(
                    m[:], ssq_sb[:, ts(G, 4)], 1.0 / HD, epsb, ALU.mult, ALU.add
                )
                rsc = wpool.tile([128, 4, 4], F32, tag="rsc")
                nc.vector.scalar_tensor_tensor(
                    rsc[:], m[:], -2.22, c26b, ALU.mult, ALU.add
                )
                nc.vector.tensor_scalar(rsc[:], rsc[:], 0.8, 2.5, ALU.max, ALU.min)
                nt = wpool.tile([128, 4, 4], F32, tag="nt")
                for _ in range(3):
                    nc.vector.tensor_mul(nt[:], rsc[:], rsc[:])
                    nc.vector.tensor_mul(nt[:], nt[:], m[:])
                    nc.vector.scalar_tensor_tensor(
                        nt[:], nt[:], -0.5, c15b, ALU.mult, ALU.add
                    )
                    nc.vector.tensor_mul(rsc[:], rsc[:], nt[:])

                # batched rotary over the 4 t-blocks (q and k separately)
                cosb = (
                    cos_sb[:, ts(G, 4), :].unsqueeze(2).broadcast_to([128, 4, 2, 32])
                )
                sinb = (
                    sin_sb[:, ts(G, 4), :].unsqueeze(2).broadcast_to([128, 4, 2, 32])
                )
                stages = []
                for qk in range(2):  # 0: q, 1: k
                    eng = nc.vector
                    raw = qkraw[:, :, ds(qk * 128, 128)].rearrange(
                        "p a (h c) -> p a h c", c=HD
                    )
                    n1 = raw[:, :, :, 0:32]
                    n2 = raw[:, :, :, 32:64]
                    st = wpool.tile([128, 4, 2, HD], FP16, tag=f"st{qk}")
                    tmp = wpool.tile([128, 4, 2, 32], FP16, tag=f"rt1{qk}")
                    tmp2 = wpool.tile([128, 4, 2, 32], FP16, tag=f"rt2{qk}")
                    eng.tensor_mul(tmp[:], n1, cosb)
                    eng.tensor_mul(tmp2[:], n2, sinb)
                    eng.tensor_add(st[:, :, :, 0:32], tmp[:], tmp2[:])
                    eng.tensor_mul(tmp[:], n2, cosb)
                    eng.tensor_mul(tmp2[:], n1, sinb)
                    eng.tensor_sub(st[:, :, :, 32:64], tmp[:], tmp2[:])
                    rscb = (
                        rsc[:, :, ds(qk * 2, 2)]
                        .unsqueeze(3)
                        .broadcast_to([128, 4, 2, HD])
                    )
                    eng.tensor_mul(st[:], st[:], rscb)
                    stages.append(st)

                # batched DMA xbar transposes: [t, 4*128] -> 4 tiles of [c, t]
                nc.sync.dma_start_transpose(
                    qT_sb[:, ts(G, 512)].rearrange("p (a b) -> p a b", a=4),
                    stages[0][:].rearrange("p a h c -> p (a h c)"),
                )
                nc.sync.dma_start_transpose(
                    kT_sb[:, ts(G, 512)].rearrange("p (a b) -> p a b", a=4),
                    stages[1][:].rearrange("p a h c -> p (a h c)"),
                )

            # ---- stage C: differential attention + partial projection ----
            # The projection of chunk tc-1 is software-pipelined into the
            # attention si-loop of chunk tc (one i-half per s-block) so its
            # matmuls and PSUM evacuations fill the exp-wait gaps.
            def emit_proj_half(pc, tb4, half, ost, pool):
                po = pool.tile([128, 512], F32, tag="pd" if pool is pd_pool else "po")
                nc.tensor.matmul(
                    po[:],
                    ycomb_sb[:, ds(pc * 512 + tb4 * 128, 128)],
                    wpp_sb[:, ts(half, 512)],
                    start=True,
                    stop=True,
                )
                nc.vector.tensor_copy(ost[:, tb4, ds(half * 512, 512)], po[:])

            def emit_chunk(tc_i):
                nsb = 4 * tc_i + 4  # s-blocks touching this t-chunk
                pyp = pyp_pool.tile([128, 1024], F32, tag="pyp", name="pyp")
                pd0 = pd_pool.tile([128, 512], F32, tag="pd", name="pd")
                if tc_i:
                    ost_prev = opool.tile([128, 4, D], FP16, tag="ost", name="ost")
                else:
                    ost_prev = None
                for si in range(nsb):
                    col0 = max(0, si * 128 - tc_i * 512)
                    w = 512 - col0
                    ppb = ppb_pool.tile([128, 1024], F32, tag="ppb")
                    for g in range(2):
                        nc.tensor.matmul(
                            ppb[:, ds(512 * g + col0, w)],
                            kT_sb[ds(g * 64, 64), ts(si, 128)],
                            qT_sb[ds(g * 64, 64), ds(tc_i * 512 + col0, w)],
                            start=True,
                            stop=True,
                        )
                    pt = ppool.tile([128, 1024], FP16, tag="pt")
                    if col0 == 0:
                        nc.scalar.activation(
                            pt[:], ppb[:], AF.Exp, bias=bn8_sb[:], scale=SCALE
                        )
                    else:
                        for g in range(2):
                            nc.scalar.activation(
                                pt[:, ds(512 * g + col0, w)],
                                ppb[:, ds(512 * g + col0, w)],
                                AF.Exp,
                                bias=bn8_sb[:],
                                scale=SCALE,
                            )
                    if col0 > 0 or si * 128 == tc_i * 512:
                        # diagonal block: zero out s > t inside it (both groups)
                        dslc = (
                            pt[:]
                            .rearrange("p (g t) -> p g t", g=2)[:, :, ds(col0, 128)]
                        )
                        nc.vector.tensor_mul(
                            dslc,
                            dslc,
                            diag_sb[:].unsqueeze(1).broadcast_to([128, 2, 128]),
                        )
                    last = si == nsb - 1
                    for g in range(2):
                        nc.tensor.matmul(
                            pyp[:, ds(512 * g + col0, w)],
                            v_sb[:, si, :],
                            pt[:, ds(512 * g + col0, w)],
                            start=(si == 0),
                            stop=last,
                        )
                    # denom g0 on PE
                    nc.tensor.matmul(
                        pd0[:, ds(col0, w)],
                        ones_sb[:],
                        pt[:, ds(col0, w)],
                        start=(si == 0),
                        stop=last,
                    )
                    # denom g1 accumulates on gpsimd (off the critical path)
                    if si == 0:
                        nc.vector.tensor_copy(dacc1_sb[:], pt[:, 512:1024])
                    else:
                        nc.gpsimd.tensor_add(
                            dacc1_sb[:, ds(col0, w)],
                            dacc1_sb[:, ds(col0, w)],
                            pt[:, ds(512 + col0, w)],
                        )
                    # pipelined projection of the previous chunk
                    if tc_i and si < 8:
                        emit_proj_half(tc_i - 1, si // 2, si % 2, ost_prev, po_pool)
                        if si == 7:
                            nc.sync.dma_start(
                                out=outd[:, ts(tc_i - 1, 4 * D)],
                                in_=ost_prev[:].rearrange("p a b -> p (a b)"),
                            )

                rec0 = wpool.tile([128, 512], F32, tag="rec0")
                nc.vector.reciprocal_approx_fast(rec0[:], pd0[:])
                pdb1 = po_pool.tile([128, 512], F32, tag="po", name="po")
                nc.tensor.matmul(
                    pdb1[:], ones_sb[:], dacc1_sb[:], start=True, stop=True
                )
                rec1 = wpool.tile([128, 512], F32, tag="rec1")
                nc.vector.reciprocal_approx_fast(rec1[:], pdb1[:])
                yn0 = wpool.tile([128, 512], F32, tag="yn0")
                nc.vector.tensor_mul(yn0[:], pyp[:, 0:512], rec0[:])
                yn1 = wpool.tile([128, 512], F32, tag="yn1")
                nc.vector.tensor_mul(yn1[:], pyp[:, 512:1024], rec1[:])
                nc.vector.scalar_tensor_tensor(
                    ycomb_sb[:, ts(tc_i, 512)],
                    yn1[:],
                    -lam,
                    yn0[:],
                    ALU.mult,
                    ALU.add,
                )

            for G in range(4):
                emit_group(G)
            for tc_i in range(NTC):
                emit_chunk(tc_i)

            # final chunk's projection (alternating the two free PSUM banks)
            ost = opool.tile([128, 4, D], FP16, tag="ost")
            for tb4 in range(4):
                for half in range(2):
                    pool = po_pool if (tb4 * 2 + half) % 2 == 0 else pd_pool
                    emit_proj_half(NTC - 1, tb4, half, ost, pool)
                nc.sync.dma_start(
                    out=outd[
                        :, (NTC - 1) * 4 * D + tb4 * D : (NTC - 1) * 4 * D + (tb4 + 1) * D
                    ],
                    in_=ost[:, tb4].rearrange("p b -> p b"),
                )

    nc.compile()
    return nc


def _make_in_maps(x, Wq, Wk, Wv, Wproj):
    f16 = np.float16
    xT = np.ascontiguousarray(x[0].T).astype(f16)  # [D, T]
    # x repack: [p, tc, kc, u] = x[0, tc*512+u, kc*128+p]
    xr = xT.reshape(KC, 128, NTC, 512).transpose(1, 2, 0, 3).reshape(128, -1)
    xr = np.ascontiguousarray(xr)

    # rotary tables, [tp, tb, 32] layout, cos|sin concatenated
    inv = 1.0 / (10000.0 ** (np.arange(0, HD, 2, dtype=np.float32) / HD))
    fr = np.outer(np.arange(T, dtype=np.float32), inv)  # [T, 32]
    cos = np.cos(fr).reshape(TB, 128, 32).transpose(1, 0, 2).reshape(128, -1)
    sin = np.sin(fr).reshape(TB, 128, 32).transpose(1, 0, 2).reshape(128, -1)
    cost = np.concatenate([cos, sin], axis=1).astype(f16)
    diag = np.triu(np.ones((128, 128), np.float32)).astype(f16)

    in_maps = []
    for h in range(NCORES):
        wcat = np.concatenate(
            [
                Wq[h * 64 : h * 64 + 64],
                Wq[512 + h * 64 : 512 + h * 64 + 64],
                Wk[h * 64 : h * 64 + 64],
                Wk[512 + h * 64 : 512 + h * 64 + 64],
                Wv[h * 128 : h * 128 + 128],
            ],
            axis=0,
        ).T  # [D, 384]
        wqkv = wcat.reshape(KC, 128, 384).transpose(1, 0, 2).reshape(128, -1)
        wpp = Wproj[:, h * 128 : (h + 1) * 128].T  # [j, i]
        in_maps.append(
            {
                "x": xr,
                "wqkv": np.ascontiguousarray(wqkv).astype(f16),
                "wpp": np.ascontiguousarray(wpp).astype(f16),
                "cost": cost,
                "diag": diag,
            }
        )
    return in_maps


def _get_program(lam: float):
    key = round(lam, 10)
    if key not in _CACHE:
        _CACHE[key] = _build_program(lam)
    return _CACHE[key]


def kernel(x, Wq, Wk, Wv, Wproj, lambda_q1, lambda_k1, lambda_q2, lambda_k2):
    x = np.asarray(x, np.float32)
    Wq, Wk = np.asarray(Wq, np.float32), np.asarray(Wk, np.float32)
    Wv, Wproj = np.asarray(Wv, np.float32), np.asarray(Wproj, np.float32)

    lam1 = float(np.exp(np.sum(np.asarray(lambda_q1) * np.asarray(lambda_k1))))
    lam2 = float(np.exp(np.sum(np.asarray(lambda_q2) * np.asarray(lambda_k2))))
    lam = lam1 - lam2 + LAMBDA_INIT

    in_maps = _make_in_maps(x, Wq, Wk, Wv, Wproj)
    nc = _get_program(lam)

    res = run_bass_kernel_spmd(nc, in_maps, list(range(NCORES)))
    acc = res.results[0]["out"].astype(np.float32)
    for h in range(1, NCORES):
        acc += res.results[h]["out"].astype(np.float32)
    y = acc.reshape(128, NTC, 4, D).transpose(1, 2, 0, 3).reshape(1, T, D)
    return np.ascontiguousarray(y)


if __name__ == "__main__":
    rng = np.random.default_rng(0)
    ins = {
        "x": rng.standard_normal((1, T, D), np.float32),
        "Wq": (rng.standard_normal((D, D)) * 0.02).astype(np.float32),
        "Wk": (rng.standard_normal((D, D)) * 0.02).astype(np.float32),
        "Wv": (rng.standard_normal((D, D)) * 0.02).astype(np.float32),
        "Wproj": (rng.standard_normal((D, D)) * 0.02).astype(np.float32),
        "lambda_q1": (rng.standard_normal(32) * 0.1).astype(np.float32),
        "lambda_k1": (rng.standard_normal(32) * 0.1).astype(np.float32),
        "lambda_q2": (rng.standard_normal(32) * 0.1).astype(np.float32),
        "lambda_k2": (rng.standard_normal(32) * 0.1).astype(np.float32),
    }
    y = kernel(**ins)
    print("kernel output", y.shape, y.dtype, float(np.abs(y).mean()))
